# revision 2
# baseline (speedup 1.0000x reference)
"""Trainium2 Bass kernel v2 for the AgentLayer GRU-with-action-memory model.

Data-parallel over batch: B=512 -> 8 cores x Bc=64. Per-core redesign vs v1:
  - gi (wih@x) computed in bulk 8-step blocks into PSUM banks; per-step gh
    matmuls accumulate into the same bank slices (skip_group_check).
  - argmax masks via is_ge against the row max (gaps verified > 2.7e-7 on
    this seed, so the max is unique); logits computed in a "doubled" layout
    [128 = 2 half-copies of b, 10 cols in per-half physical-slot order].
  - gather = one DVE/Pool masked-multiply over the ring
    [(slot%2)*64+b, h, slot//2] + 5 accumulating PE matmuls against a
    constant tiled-identity E2, producing weighted_h' = 4*weighted_h in
    [H, Bc] PSUM directly.  whh is pre-divided by 4 (exact) to compensate.
  - All matmuls strictly fp32 (fp32r/bf16 flip argmax decisions; verified).
"""

import numpy as np
from contextlib import ExitStack

B, T, D, H, S, A, U = 512, 256, 256, 128, 64, 10, 64
NCORES = 8
BC = B // NCORES  # 64
G3 = 3 * H        # 384

_BUILD_CACHE = {}


def _phys(j, upper):
    if j < 5:
        return 2 * j + (1 if upper else 0)
    return 2 * (j - 5) + (0 if upper else 1)


def _build(Tn, Bc):
    key = (Tn, Bc)
    if key in _BUILD_CACHE:
        return _BUILD_CACHE[key]

    import concourse.bass as bass
    import concourse.bacc as bacc
    import concourse.tile as tile
    from concourse import mybir

    f32 = mybir.dt.float32
    Alu = mybir.AluOpType
    Act = mybir.ActivationFunctionType
    Axis = mybir.AxisListType

    nc = bacc.Bacc("TRN2", target_bir_lowering=False, debug=False)

    d_in = {}

    def din(name, shape):
        d_in[name] = nc.dram_tensor(name, list(shape), f32, kind="ExternalInput").ap()
        return d_in[name]

    xT = din("xT", (2, 128, Tn, Bc))          # x as [chunk, d, t, b]
    staticT = din("staticT", (S, Bc))
    static_rep = din("static_rep", (S, 8 * Bc))
    wihT = din("wihT", (2, 128, G3))
    whh4T = din("whh4T", (H, G3))             # (whh/4).T
    w1h10T = din("w1h10T", (H, U))            # (a1_w1[:, :H]/10).T
    w1h10Tn = din("w1h10Tn", (H, U))          # negated
    w1sT = din("w1sT", (S, U))
    a2w1xT = din("a2w1xT", (2, 128, U))
    a2w1sT = din("a2w1sT", (S, U))
    w2re1 = din("w2re1", (10, 2, U, A))       # a1_w2.T column-permuted per t%10, half
    w2re2 = din("w2re2", (10, 2, U, A))       # a2_w2.T likewise
    b2re1 = din("b2re1", (10, 2, A))          # a1_b2 permuted rows
    b2re2 = din("b2re2", (10, 2, A))
    deltas = din("deltas", (5, Bc, 8 * A))    # phase-A newest-slot marks, col-doubled
    initwT = din("initwT", (S, H))
    fuswhT = din("fuswhT", (H, H))
    fuswsT = din("fuswsT", (S, H))
    a1b1 = din("a1b1", (U, 1))
    a2b1 = din("a2b1", (U, 1))
    brz_row = din("brz_row", (1, 2 * H))      # bih+bhh for r,z gates
    bin_row = din("bin_row", (1, H))          # bih n-gate
    bhn_row = din("bhn_row", (1, H))          # bhh n-gate
    ones_row = din("ones_row", (1, 8 * Bc))
    fusb_row = din("fusb_row", (1, H))
    initb_row = din("initb_row", (1, H))
    E2 = din("E2", (2 * Bc, Bc))              # tiled identity
    SPL = din("SPL", (Bc, 2 * Bc))            # [eye | 0] spread-to-lower
    SPU = din("SPU", (Bc, 2 * Bc))            # [0 | eye] spread-to-upper
    ident = din("ident", (128, 128))

    out_d = nc.dram_tensor("out", [H, Tn, Bc], f32, kind="ExternalOutput").ap()

    NBLK = Tn // 8
    assert Tn % 8 == 0
    OUT_BLK = 16

    with ExitStack() as ctx:
        tc = ctx.enter_context(tile.TileContext(nc))
        singles = ctx.enter_context(tc.tile_pool(name="singles", bufs=1))
        work = ctx.enter_context(tc.tile_pool(name="work", bufs=3))
        psg = ctx.enter_context(tc.tile_pool(name="psg", bufs=2, space="PSUM"))     # gi banks (3/blk)
        psfix = ctx.enter_context(tc.tile_pool(name="psfix", bufs=1, space="PSUM"))  # 2 packed banks
        u2t_pool = ctx.enter_context(tc.tile_pool(name="u2t", bufs=2))
        pre2_pool = ctx.enter_context(tc.tile_pool(name="pre2", bufs=4))
        out_sb_pool = ctx.enter_context(tc.tile_pool(name="outsb", bufs=2))

        # ---- load constants / weights into SBUF ----
        sb = {}
        for name, ap in d_in.items():
            if name == "xT":
                continue
            if name in ("wihT", "a2w1xT"):
                t = singles.tile([128, 2, ap.shape[2]], f32, tag=f"w_{name}", name=f"w_{name}")
                for c in range(2):
                    nc.sync.dma_start(out=t[:, c, :], in_=ap[c])
            elif name in ("w2re1", "w2re2"):
                t = singles.tile([U, 10, 2, A], f32, tag=f"w_{name}", name=f"w_{name}")
                nc.sync.dma_start(out=t, in_=bass.AP(
                    tensor=ap.tensor, offset=ap.offset,
                    ap=[[ap.ap[2][0], U], [ap.ap[0][0], 10], [ap.ap[1][0], 2], [ap.ap[3][0], A]]))
            elif name in ("b2re1", "b2re2"):
                t = singles.tile([1, 10, 2, A], f32, tag=f"w_{name}", name=f"w_{name}")
                nc.sync.dma_start(out=t, in_=bass.AP(
                    tensor=ap.tensor, offset=ap.offset,
                    ap=[[0, 1], [ap.ap[0][0], 10], [ap.ap[1][0], 2], [ap.ap[2][0], A]]))
            elif name == "deltas":
                t = singles.tile([Bc, 5, 8 * A], f32, tag="w_deltas", name="w_deltas")
                nc.sync.dma_start(out=t, in_=bass.AP(
                    tensor=ap.tensor, offset=ap.offset,
                    ap=[[ap.ap[1][0], Bc], [ap.ap[0][0], 5], [ap.ap[2][0], 8 * A]]))
            else:
                t = singles.tile(list(ap.shape), f32, tag=f"w_{name}", name=f"w_{name}")
                nc.sync.dma_start(out=t, in_=ap)
            sb[name] = t

        xsb = singles.tile([128, 2, Tn, Bc], f32, tag="xsb")
        for c in range(2):
            nc.sync.dma_start(out=xsb[:, c, :, :], in_=xT[c])

        # ---- persistent state ----
        ring2 = singles.tile([2 * Bc, 5, H], f32, tag="ring2")   # [(s%2)*64+b, s//2, h]
        ring_m = singles.tile([H, A, Bc], f32, tag="ring_m")     # [h, slot, b]
        rsum = singles.tile([H, Bc], f32, tag="rsum")
        nc.vector.memset(ring2, 0.0)
        nc.vector.memset(ring_m, 0.0)
        nc.vector.memset(rsum, 0.0)

        pre2_tiles = {}

        # two packed psum banks, manually partitioned by column range
        psA_t = psfix.tile([128, 512], f32, tag="psA")
        psB_t = psfix.tile([128, 512], f32, tag="psB")
        pu2_sl = psA_t[0:U, 0:256]
        pl2_sl = psA_t[0:Bc, 256:336].rearrange("p (t a) -> p t a", a=2 * A)
        ptr_sl = psA_t[0:Bc, 336:464]
        pu1_sl = psB_t[0:U, 0:Bc]
        pl1_sl = psB_t[0:Bc, 64:64 + 2 * A]
        cmd_sl = psB_t[:, 88:93]
        pwh_sl = psB_t[:, 128:192]
        phn_sl = psB_t[:, 192:256]
        pout_sl = psB_t[:, 256:320]
        pfs_sl = psB_t[:, 320:384]
        pinit_sl = psB_t[:, 384:448]

        # ---------- phase A: pre2_d for a 4-step block ----------
        PA_BLK = 4
        def emit_phaseA(blk):
            t0 = blk * PA_BLK
            NF = PA_BLK * Bc
            pu2 = pu2_sl
            for c in range(2):
                nc.tensor.matmul(pu2, sb["a2w1xT"][:, c, :],
                                 xsb[:, c, t0:t0 + PA_BLK, :].rearrange("p t b -> p (t b)"),
                                 start=(c == 0), stop=False)
            nc.tensor.matmul(pu2, sb["a2w1sT"], sb["static_rep"][:, 0:NF],
                             start=False, stop=True)
            u2t = u2t_pool.tile([U, NF], f32, tag="u2t")
            nc.scalar.activation(u2t, pu2, Act.Tanh, bias=sb["a2b1"], scale=1.0)
            pl2 = pl2_sl
            for ti in range(PA_BLK):
                tt = (t0 + ti) % 10
                for half in range(2):
                    nc.tensor.matmul(pl2[:, ti, half * A:(half + 1) * A],
                                     sb["ones_row"][:, 0:Bc], sb["b2re2"][:, tt, half, :],
                                     start=(half == 0), stop=False)
                    nc.tensor.matmul(pl2[:, ti, half * A:(half + 1) * A],
                                     u2t[:, ti * Bc:(ti + 1) * Bc],
                                     sb["w2re2"][:, tt, half, :],
                                     start=False, stop=(half == 1))
            rmax2 = work.tile([Bc, PA_BLK], f32, tag="rmax2")
            nc.vector.tensor_reduce(out=rmax2, in_=pl2, axis=Axis.X, op=Alu.max)
            rmax2_b = bass.AP(tensor=rmax2.tensor, offset=rmax2.offset,
                              ap=[rmax2.ap[0], rmax2.ap[1], [0, 2 * A]])
            ge2 = work.tile([Bc, PA_BLK, 2 * A], f32, tag="ge2")
            nc.vector.tensor_tensor(out=ge2, in0=pl2, in1=rmax2_b, op=Alu.is_ge)
            pre2 = pre2_pool.tile([Bc, PA_BLK, 2 * A], f32, tag="pre2", name=f"pre2_{blk}")
            nc.gpsimd.tensor_tensor(
                out=pre2, in0=ge2,
                in1=sb["deltas"][:, blk % 5, :].rearrange("p (t a) -> p t a", a=2 * A),
                op=Alu.add)
            pre2_tiles[blk] = pre2

        # ---------- bulk gi for a 4-step block (GI_BLK=4 keeps each psum
        # tile within one 2KB bank) ----------
        GI_BLK = 4
        gi_banks = {}

        def emit_gi_block(gblk):
            t0 = gblk * GI_BLK
            NF = GI_BLK * Bc
            gr = psg.tile([128, GI_BLK, Bc], f32, tag="gir", name=f"gir_{gblk}")
            gz = psg.tile([128, GI_BLK, Bc], f32, tag="giz", name=f"giz_{gblk}")
            gn = psg.tile([128, GI_BLK, Bc], f32, tag="gin", name=f"gin_{gblk}")
            for gate, tile_ in ((0, gr), (1, gz), (2, gn)):
                flat = tile_.rearrange("p t b -> p (t b)")
                brow = (sb["brz_row"][:, gate * H:(gate + 1) * H] if gate < 2
                        else sb["bin_row"])
                nc.tensor.matmul(flat, brow, sb["ones_row"][:, 0:NF],
                                 start=True, stop=False, skip_group_check=True)
                for c in range(2):
                    nc.tensor.matmul(flat, sb["wihT"][:, c, gate * H:(gate + 1) * H],
                                     xsb[:, c, t0:t0 + GI_BLK, :].rearrange("p t b -> p (t b)"),
                                     start=False, stop=False, skip_group_check=True)
            gi_banks[gblk] = (gr, gz, gn)

        # ---------- per-step tail: gh, gates, fusion ----------
        out_tiles = {}
        fus_static = singles.tile([H, Bc], f32, tag="fus_static")

        def emit_step_tail(t, whp, whp_sb):
            """whp: psum (or sbuf) [H, Bc] holding 4*weighted_h; whp_sb: sbuf copy."""
            gblk, ti = t // GI_BLK, t % GI_BLK
            gr, gz, gn = gi_banks[gblk]
            # gh accumulation into gi slices
            nc.tensor.matmul(gr[:, ti, :], sb["whh4T"][:, 0:H], whp_sb,
                             start=False, stop=True, skip_group_check=True)
            nc.tensor.matmul(gz[:, ti, :], sb["whh4T"][:, H:2 * H], whp_sb,
                             start=False, stop=True, skip_group_check=True)
            phn = phn_sl
            nc.tensor.matmul(phn, sb["bhn_row"], sb["ones_row"][:, 0:Bc], start=True, stop=False)
            nc.tensor.matmul(phn, sb["whh4T"][:, 2 * H:3 * H], whp_sb, start=False, stop=True)
            thr = work.tile([H, Bc], f32, tag="thr")
            nc.scalar.activation(thr, gr[:, ti, :], Act.Tanh, bias=0.0, scale=0.5)
            thz = work.tile([H, Bc], f32, tag="thz")
            nc.scalar.activation(thz, gz[:, ti, :], Act.Tanh, bias=0.0, scale=0.5)
            q = work.tile([H, Bc], f32, tag="q")
            nc.vector.scalar_tensor_tensor(out=q, in0=thr, scalar=1.0, in1=phn,
                                           op0=Alu.add, op1=Alu.mult)
            pren = work.tile([H, Bc], f32, tag="pren")
            nc.vector.scalar_tensor_tensor(out=pren, in0=q, scalar=0.5, in1=gn[:, ti, :],
                                           op0=Alu.mult, op1=Alu.add)
            thn = work.tile([H, Bc], f32, tag="thn")
            nc.scalar.activation(thn, pren, Act.Tanh, bias=0.0, scale=1.0)
            dgf = work.tile([H, Bc], f32, tag="dgf")
            nc.vector.scalar_tensor_tensor(out=dgf, in0=whp, scalar=0.25, in1=thn,
                                           op0=Alu.mult, op1=Alu.subtract)
            e = work.tile([H, Bc], f32, tag="e")
            nc.vector.scalar_tensor_tensor(out=e, in0=thz, scalar=1.0, in1=dgf,
                                           op0=Alu.add, op1=Alu.mult)
            new_h = work.tile([H, Bc], f32, tag="new_h")
            nc.vector.scalar_tensor_tensor(out=new_h, in0=e, scalar=0.5, in1=thn,
                                           op0=Alu.mult, op1=Alu.add)
            # fusion output
            pout = pout_sl
            nc.tensor.matmul(pout, sb["fuswhT"], new_h, start=True, stop=True)
            ob = t // OUT_BLK
            if ob not in out_tiles:
                out_tiles[ob] = out_sb_pool.tile([H, OUT_BLK, Bc], f32, tag="osb", name=f"osb_{ob}")
            nc.vector.tensor_tensor(out=out_tiles[ob][:, t % OUT_BLK, :], in0=pout,
                                     in1=fus_static, op=Alu.add)
            if t % OUT_BLK == OUT_BLK - 1:
                nc.sync.dma_start(out=out_d[:, t - OUT_BLK + 1:t + 1, :], in_=out_tiles[ob])
                del out_tiles[ob]
            return new_h

        # ---------- prologue ----------
        emit_phaseA(0)
        emit_phaseA(1)
        emit_phaseA(2)
        emit_gi_block(0)
        emit_gi_block(1)

        # fus_static = fuswsT@staticT + fusb
        pfs = pfs_sl
        nc.tensor.matmul(pfs, sb["fusb_row"], sb["ones_row"][:, 0:Bc], start=True, stop=False)
        nc.tensor.matmul(pfs, sb["fuswsT"], sb["staticT"], start=False, stop=True)
        nc.scalar.copy(fus_static, pfs)

        # t = 0: wh' = 4*cur0
        pinit = pinit_sl
        nc.tensor.matmul(pinit, sb["initb_row"], sb["ones_row"][:, 0:Bc], start=True, stop=False)
        nc.tensor.matmul(pinit, sb["initwT"], sb["staticT"], start=False, stop=True)
        def emit_maintenance(t):
            """After new_h_t: pu1_pre(t+1), rsum update, ring writes."""
            s2 = t % A
            if t + 1 < Tn:
                nc.tensor.matmul(pu1_sl, sb["w1sT"], sb["staticT"], start=True, stop=False)
                nc.tensor.matmul(pu1_sl, sb["w1h10T"], rsum, start=False, stop=False)
            nc.gpsimd.tensor_tensor(out=rsum, in0=rsum, in1=cur_h, op=Alu.add)
            s3 = (t + 1) % A
            if t + 1 < Tn:
                nc.gpsimd.tensor_tensor(out=rsum, in0=rsum, in1=ring_m[:, s3, :],
                                        op=Alu.subtract)
            nc.gpsimd.tensor_copy(ring_m[:, s2, :], cur_h)
            half2, sq2w = s2 % 2, s2 // 2
            nc.tensor.matmul(ptr_sl, cur_h, sb["ident"],
                             is_transpose=True, start=True, stop=True)
            if half2 == 0:
                nc.scalar.copy(ring2[0:Bc, sq2w, :], ptr_sl)
            else:
                hT = work.tile([Bc, H], f32, tag="hT")
                nc.scalar.copy(hT, ptr_sl)
                nc.sync.dma_start(out=ring2[Bc:2 * Bc, sq2w, :], in_=hT)

        wh0 = work.tile([H, Bc], f32, tag="wh0")
        nc.vector.tensor_scalar(out=wh0, in0=pinit, scalar1=4.0, scalar2=None, op0=Alu.mult)
        cur_h = emit_step_tail(0, wh0, wh0)
        emit_maintenance(0)

        # ---------- scan t = 1..Tn-1 ----------
        for t in range(1, Tn):
            blk, ti = t // 4, t % 4
            s = (t - 1) % A
            tt = t % 10

            # on-path: close the pu1 accumulation with the h_{t-1} term
            pu1 = pu1_sl
            nc.tensor.matmul(pu1, sb["w1h10T"], cur_h, start=False, stop=True)
            u1t = work.tile([U, Bc], f32, tag="u1t")
            nc.scalar.activation(u1t, pu1, Act.Tanh, bias=sb["a1b1"], scale=1.0)

            # logits1 in col-doubled layout [64, 20]; both 10-blocks hold all
            # 10 actions (permuted), so one row-max covers both.
            pl1 = pl1_sl
            for hf in range(2):
                nc.tensor.matmul(pl1[:, hf * A:(hf + 1) * A], sb["ones_row"][:, 0:Bc],
                                 sb["b2re1"][:, tt, hf, :], start=(hf == 0), stop=False)
                nc.tensor.matmul(pl1[:, hf * A:(hf + 1) * A], u1t,
                                 sb["w2re1"][:, tt, hf, :], start=False, stop=(hf == 1))
            rmax1 = work.tile([Bc, 1], f32, tag="rmax1")
            nc.vector.tensor_reduce(out=rmax1, in_=pl1, axis=Axis.X, op=Alu.max)
            cmask = work.tile([Bc, 2 * A], f32, tag="cmask")
            nc.vector.scalar_tensor_tensor(out=cmask, in0=pl1, scalar=rmax1[:, 0:1],
                                           in1=pre2_tiles[blk][:, ti, :],
                                           op0=Alu.is_ge, op1=Alu.add)
            # spread the per-half gather columns to partitions (half, b)
            cmd = cmd_sl
            nc.tensor.matmul(cmd, sb["SPL"], cmask[:, 0:5], start=True, stop=False)
            nc.tensor.matmul(cmd, sb["SPU"], cmask[:, A:A + 5], start=False, stop=True)
            # D[p, (s', b)] = cmd[p, s'] * eye2[p, b], then
            # wh'[h, b] = sum_s' ring2[:, :, s'].T @ D[:, s', :]
            Dm = work.tile([2 * Bc, 5, Bc], f32, tag="Dm")
            cm_b = bass.AP(tensor=cmd.tensor, offset=cmd.offset,
                           ap=[cmd.ap[0], [cmd.ap[1][0], 5], [0, Bc]])
            e2_b = bass.AP(tensor=sb["E2"].tensor, offset=sb["E2"].offset,
                           ap=[sb["E2"].ap[0], [0, 5], [sb["E2"].ap[1][0], Bc]])
            nc.vector.tensor_tensor(out=Dm, in0=cm_b, in1=e2_b, op=Alu.mult)
            pwh = pwh_sl
            for sq2 in range(5):
                nc.tensor.matmul(pwh, ring2[:, sq2, :], Dm[:, sq2, :],
                                 start=(sq2 == 0), stop=(sq2 == 4))
            whp_sb = work.tile([H, Bc], f32, tag="whp_sb")
            nc.vector.tensor_copy(whp_sb, pwh)

            cur_h = emit_step_tail(t, pwh, whp_sb)
            emit_maintenance(t)
            # bulk blocks emitted after the chain so they fill PE gaps
            if ti == 1:
                if (blk + 1) * GI_BLK < Tn:
                    emit_gi_block(blk + 1)
                if (blk + 2) * PA_BLK < Tn:
                    emit_phaseA(blk + 2)
                if blk - 2 in pre2_tiles:
                    del pre2_tiles[blk - 2]

    nc.compile()
    _BUILD_CACHE[key] = (nc, "out")
    return _BUILD_CACHE[key]


def _prep_core_inputs(inputs, core, Tn=T, Bc=BC):
    f = np.float32
    b0 = core * Bc
    x = np.ascontiguousarray(inputs["x"][b0:b0 + Bc, :Tn, :]).astype(f)
    xT = np.ascontiguousarray(x.transpose(2, 1, 0).reshape(2, 128, Tn, Bc))
    staticT = np.ascontiguousarray(inputs["static"][b0:b0 + Bc].T).astype(f)
    wih = inputs["gru_wih"].astype(f); whh = inputs["gru_whh"].astype(f)
    a1w1 = inputs["a1_w1"].astype(f); a2w1 = inputs["a2_w1"].astype(f)
    bih = inputs["gru_bih"].astype(f); bhh = inputs["gru_bhh"].astype(f)
    w2_1 = inputs["a1_w2"].astype(f); w2_2 = inputs["a2_w2"].astype(f)
    b2_1 = inputs["a1_b2"].astype(f); b2_2 = inputs["a2_b2"].astype(f)

    # permuted second-layer weights: col j of variant (tt, half) = row a of w2
    # with a = (phys(j, half) - tt) mod 10
    w2re1 = np.zeros((10, 2, U, A), f); w2re2 = np.zeros((10, 2, U, A), f)
    b2re1 = np.zeros((10, 2, A), f); b2re2 = np.zeros((10, 2, A), f)
    for tt in range(10):
        for hf in range(2):
            for j in range(A):
                a = (_phys(j, hf) - tt) % 10
                w2re1[tt, hf, :, j] = w2_1[a, :]
                w2re2[tt, hf, :, j] = w2_2[a, :]
                b2re1[tt, hf, j] = b2_1[a]
                b2re2[tt, hf, j] = b2_2[a]

    # deltas: per phase-A block variant v (t0%10 = 2v cycle {0,8,6,4,2}),
    # 8 step-columns; each step t marks column j where phys(j, half) == (t-1)%10
    # with 2.0 (the 0.5*cur_h direct term, x4 scale)
    deltas = np.zeros((5, Bc, 4 * 2 * A), f)
    for blk_v in range(5):
        t0mod = (blk_v * 4) % 10
        for ti in range(4):
            tmod = (t0mod + ti) % 10
            s9 = (tmod - 1) % 10
            for hf in range(2):
                for j in range(A):
                    if _phys(j, hf) == s9:
                        deltas[blk_v, :, ti * 2 * A + hf * A + j] = 2.0

    m = {
        "xT": xT,
        "staticT": staticT,
        "static_rep": np.tile(staticT, (1, 8)),
        "wihT": np.ascontiguousarray(wih.T.reshape(2, 128, G3)),
        "whh4T": np.ascontiguousarray((whh / 4.0).T),
        "w1h10T": np.ascontiguousarray((a1w1[:, :H] / 10.0).T),
        "w1h10Tn": np.ascontiguousarray((-a1w1[:, :H] / 10.0).T),
        "w1sT": np.ascontiguousarray(a1w1[:, H:].T),
        "a2w1xT": np.ascontiguousarray(a2w1[:, :D].T.reshape(2, 128, U)),
        "a2w1sT": np.ascontiguousarray(a2w1[:, D:].T),
        "w2re1": w2re1, "w2re2": w2re2, "b2re1": b2re1, "b2re2": b2re2,
        "deltas": deltas,
        "initwT": np.ascontiguousarray(inputs["init_w"].astype(f).T),
        "fuswhT": np.ascontiguousarray(inputs["fus_w"].astype(f)[:, :H].T),
        "fuswsT": np.ascontiguousarray(inputs["fus_w"].astype(f)[:, H:].T),
        "a1b1": inputs["a1_b1"].astype(f).reshape(U, 1),
        "a2b1": inputs["a2_b1"].astype(f).reshape(U, 1),
        "brz_row": (bih[:2 * H] + bhh[:2 * H]).reshape(1, 2 * H).copy(),
        "bin_row": bih[2 * H:].reshape(1, H).copy(),
        "bhn_row": bhh[2 * H:].reshape(1, H).copy(),
        "ones_row": np.ones((1, 8 * Bc), f),
        "fusb_row": inputs["fus_b"].astype(f).reshape(1, H).copy(),
        "initb_row": inputs["init_b"].astype(f).reshape(1, H).copy(),
        "E2": np.tile(np.eye(Bc, dtype=f), (2, 1)),
        "SPL": np.concatenate([np.eye(Bc, dtype=f), np.zeros((Bc, Bc), f)], axis=1),
        "SPU": np.concatenate([np.zeros((Bc, Bc), f), np.eye(Bc, dtype=f)], axis=1),
        "ident": np.eye(128, dtype=f),
    }
    return {k: np.ascontiguousarray(v, dtype=f) for k, v in m.items()}


def kernel(**inputs):
    from concourse.bass_utils import run_bass_kernel_spmd
    nc, _ = _build(T, BC)
    in_maps = [_prep_core_inputs(inputs, c) for c in range(NCORES)]
    res = run_bass_kernel_spmd(nc, in_maps, core_ids=list(range(NCORES)))
    out = np.empty((B, T, H), np.float32)
    for c in range(NCORES):
        oc = res.results[c]["out"]
        out[c * BC:(c + 1) * BC] = oc.transpose(2, 1, 0)
    return out


# revision 4
# speedup vs baseline: 1.1264x; 1.1264x over previous
"""Trainium2 Bass kernel v2 for the AgentLayer GRU-with-action-memory model.

Data-parallel over batch: B=512 -> 8 cores x Bc=64. Per-core redesign vs v1:
  - gi (wih@x) computed in bulk 8-step blocks into PSUM banks; per-step gh
    matmuls accumulate into the same bank slices (skip_group_check).
  - argmax masks via is_ge against the row max (gaps verified > 2.7e-7 on
    this seed, so the max is unique); logits computed in a "doubled" layout
    [128 = 2 half-copies of b, 10 cols in per-half physical-slot order].
  - gather = one DVE/Pool masked-multiply over the ring
    [(slot%2)*64+b, h, slot//2] + 5 accumulating PE matmuls against a
    constant tiled-identity E2, producing weighted_h' = 4*weighted_h in
    [H, Bc] PSUM directly.  whh is pre-divided by 4 (exact) to compensate.
  - All matmuls strictly fp32 (fp32r/bf16 flip argmax decisions; verified).
"""

import numpy as np
from contextlib import ExitStack

B, T, D, H, S, A, U = 512, 256, 256, 128, 64, 10, 64
NCORES = 8
BC = B // NCORES  # 64
G3 = 3 * H        # 384

_BUILD_CACHE = {}


def _phys(j, upper):
    if j < 5:
        return 2 * j + (1 if upper else 0)
    return 2 * (j - 5) + (0 if upper else 1)


def _build(Tn, Bc):
    key = (Tn, Bc)
    if key in _BUILD_CACHE:
        return _BUILD_CACHE[key]

    import concourse.bass as bass
    import concourse.bacc as bacc
    import concourse.tile as tile
    from concourse import mybir

    f32 = mybir.dt.float32
    Alu = mybir.AluOpType
    Act = mybir.ActivationFunctionType
    Axis = mybir.AxisListType

    nc = bacc.Bacc("TRN2", target_bir_lowering=False, debug=False)

    d_in = {}

    def din(name, shape):
        d_in[name] = nc.dram_tensor(name, list(shape), f32, kind="ExternalInput").ap()
        return d_in[name]

    xT = din("xT", (2, 128, Tn, Bc))          # x as [chunk, d, t, b]
    staticT = din("staticT", (S, Bc))
    static_rep = din("static_rep", (S, 8 * Bc))
    wihT = din("wihT", (2, 128, G3))
    whh4T = din("whh4T", (H, G3))             # (whh/4).T
    w1h10T = din("w1h10T", (H, U))            # (a1_w1[:, :H]/10).T
    w1h10Tn = din("w1h10Tn", (H, U))          # negated
    w1sT = din("w1sT", (S, U))
    a2w1xT = din("a2w1xT", (2, 128, U))
    a2w1sT = din("a2w1sT", (S, U))
    w2re1 = din("w2re1", (10, 2, U, A))       # a1_w2.T column-permuted per t%10, half
    w2re2 = din("w2re2", (10, 2, U, A))       # a2_w2.T likewise
    b2re1 = din("b2re1", (10, 2, A))          # a1_b2 permuted rows
    b2re2 = din("b2re2", (10, 2, A))
    deltas = din("deltas", (5, Bc, 8 * A))    # phase-A newest-slot marks, col-doubled
    initwT = din("initwT", (S, H))
    fuswhT = din("fuswhT", (H, H))
    fuswsT = din("fuswsT", (S, H))
    a1b1 = din("a1b1", (U, 1))
    a2b1 = din("a2b1", (U, 1))
    brz_row = din("brz_row", (1, 2 * H))      # bih+bhh for r,z gates
    bin_row = din("bin_row", (1, H))          # bih n-gate
    bhn_row = din("bhn_row", (1, H))          # bhh n-gate
    ones_row = din("ones_row", (1, 8 * Bc))
    fusb_row = din("fusb_row", (1, H))
    initb_row = din("initb_row", (1, H))
    E2 = din("E2", (2 * Bc, Bc))              # tiled identity
    SPL = din("SPL", (Bc, 2 * Bc))            # [eye | 0] spread-to-lower
    SPU = din("SPU", (Bc, 2 * Bc))            # [0 | eye] spread-to-upper
    ident = din("ident", (128, 128))

    out_d = nc.dram_tensor("out", [H, Tn, Bc], f32, kind="ExternalOutput").ap()

    NBLK = Tn // 8
    assert Tn % 8 == 0
    OUT_BLK = 16

    with ExitStack() as ctx:
        tc = ctx.enter_context(tile.TileContext(nc))
        singles = ctx.enter_context(tc.tile_pool(name="singles", bufs=1))
        work = ctx.enter_context(tc.tile_pool(name="work", bufs=3))
        psg = ctx.enter_context(tc.tile_pool(name="psg", bufs=2, space="PSUM"))     # gi banks (3/blk)
        psfix = ctx.enter_context(tc.tile_pool(name="psfix", bufs=1, space="PSUM"))  # 2 packed banks
        u2t_pool = ctx.enter_context(tc.tile_pool(name="u2t", bufs=2))
        pre2_pool = ctx.enter_context(tc.tile_pool(name="pre2", bufs=4))
        out_sb_pool = ctx.enter_context(tc.tile_pool(name="outsb", bufs=2))

        # ---- load constants / weights into SBUF ----
        sb = {}
        for name, ap in d_in.items():
            if name == "xT":
                continue
            if name in ("wihT", "a2w1xT"):
                t = singles.tile([128, 2, ap.shape[2]], f32, tag=f"w_{name}", name=f"w_{name}")
                for c in range(2):
                    nc.sync.dma_start(out=t[:, c, :], in_=ap[c])
            elif name in ("w2re1", "w2re2"):
                t = singles.tile([U, 10, 2, A], f32, tag=f"w_{name}", name=f"w_{name}")
                nc.sync.dma_start(out=t, in_=bass.AP(
                    tensor=ap.tensor, offset=ap.offset,
                    ap=[[ap.ap[2][0], U], [ap.ap[0][0], 10], [ap.ap[1][0], 2], [ap.ap[3][0], A]]))
            elif name in ("b2re1", "b2re2"):
                t = singles.tile([1, 10, 2, A], f32, tag=f"w_{name}", name=f"w_{name}")
                nc.sync.dma_start(out=t, in_=bass.AP(
                    tensor=ap.tensor, offset=ap.offset,
                    ap=[[0, 1], [ap.ap[0][0], 10], [ap.ap[1][0], 2], [ap.ap[2][0], A]]))
            elif name == "deltas":
                t = singles.tile([Bc, 5, 8 * A], f32, tag="w_deltas", name="w_deltas")
                nc.sync.dma_start(out=t, in_=bass.AP(
                    tensor=ap.tensor, offset=ap.offset,
                    ap=[[ap.ap[1][0], Bc], [ap.ap[0][0], 5], [ap.ap[2][0], 8 * A]]))
            else:
                t = singles.tile(list(ap.shape), f32, tag=f"w_{name}", name=f"w_{name}")
                nc.sync.dma_start(out=t, in_=ap)
            sb[name] = t

        xsb = singles.tile([128, 2, Tn, Bc], f32, tag="xsb")
        for c in range(2):
            nc.sync.dma_start(out=xsb[:, c, :, :], in_=xT[c])

        # ---- persistent state ----
        ring2 = singles.tile([2 * Bc, 5, H], f32, tag="ring2")   # [(s%2)*64+b, s//2, h]
        ring_m = singles.tile([H, A, Bc], f32, tag="ring_m")     # [h, slot, b]
        rsum = singles.tile([H, Bc], f32, tag="rsum")
        nc.vector.memset(ring2, 0.0)
        nc.vector.memset(ring_m, 0.0)
        nc.vector.memset(rsum, 0.0)

        pre2_tiles = {}

        # two packed psum banks, manually partitioned by column range
        psA_t = psfix.tile([128, 512], f32, tag="psA")
        psB_t = psfix.tile([128, 512], f32, tag="psB")
        pu2_sl = psA_t[0:U, 0:256]
        pl2_sl = psA_t[0:Bc, 256:336].rearrange("p (t a) -> p t a", a=2 * A)
        ptr_sl = psA_t[0:Bc, 336:464]
        pu1_sl = psB_t[0:U, 0:Bc]
        pl1_sl = psB_t[0:Bc, 64:64 + 2 * A]
        cmd_sl = psB_t[:, 88:93]
        pwh_sl = psB_t[:, 128:192]
        phn_sl = psB_t[:, 192:256]
        pout_sl = psB_t[:, 256:320]
        pfs_sl = psB_t[:, 320:384]
        pinit_sl = psB_t[:, 384:448]

        # ---------- phase A: pre2_d for a 4-step block ----------
        PA_BLK = 4
        def emit_phaseA(blk):
            t0 = blk * PA_BLK
            NF = PA_BLK * Bc
            pu2 = pu2_sl
            for c in range(2):
                nc.tensor.matmul(pu2, sb["a2w1xT"][:, c, :],
                                 xsb[:, c, t0:t0 + PA_BLK, :].rearrange("p t b -> p (t b)"),
                                 start=(c == 0), stop=False)
            nc.tensor.matmul(pu2, sb["a2w1sT"], sb["static_rep"][:, 0:NF],
                             start=False, stop=True)
            u2t = u2t_pool.tile([U, NF], f32, tag="u2t")
            nc.scalar.activation(u2t, pu2, Act.Tanh, bias=sb["a2b1"], scale=1.0)
            pl2 = pl2_sl
            for ti in range(PA_BLK):
                tt = (t0 + ti) % 10
                for half in range(2):
                    nc.tensor.matmul(pl2[:, ti, half * A:(half + 1) * A],
                                     sb["ones_row"][:, 0:Bc], sb["b2re2"][:, tt, half, :],
                                     start=(half == 0), stop=False)
                    nc.tensor.matmul(pl2[:, ti, half * A:(half + 1) * A],
                                     u2t[:, ti * Bc:(ti + 1) * Bc],
                                     sb["w2re2"][:, tt, half, :],
                                     start=False, stop=(half == 1))
            rmax2 = work.tile([Bc, PA_BLK], f32, tag="rmax2")
            nc.vector.tensor_reduce(out=rmax2, in_=pl2, axis=Axis.X, op=Alu.max)
            rmax2_b = bass.AP(tensor=rmax2.tensor, offset=rmax2.offset,
                              ap=[rmax2.ap[0], rmax2.ap[1], [0, 2 * A]])
            ge2 = work.tile([Bc, PA_BLK, 2 * A], f32, tag="ge2")
            nc.vector.tensor_tensor(out=ge2, in0=pl2, in1=rmax2_b, op=Alu.is_ge)
            pre2 = pre2_pool.tile([Bc, PA_BLK, 2 * A], f32, tag="pre2", name=f"pre2_{blk}")
            nc.gpsimd.tensor_tensor(
                out=pre2, in0=ge2,
                in1=sb["deltas"][:, blk % 5, :].rearrange("p (t a) -> p t a", a=2 * A),
                op=Alu.add)
            pre2_tiles[blk] = pre2

        # ---------- bulk gi for a 4-step block (GI_BLK=4 keeps each psum
        # tile within one 2KB bank) ----------
        GI_BLK = 4
        gi_banks = {}

        def emit_gi_block(gblk):
            t0 = gblk * GI_BLK
            NF = GI_BLK * Bc
            gr = psg.tile([128, GI_BLK, Bc], f32, tag="gir", name=f"gir_{gblk}")
            gz = psg.tile([128, GI_BLK, Bc], f32, tag="giz", name=f"giz_{gblk}")
            gn = psg.tile([128, GI_BLK, Bc], f32, tag="gin", name=f"gin_{gblk}")
            for gate, tile_ in ((0, gr), (1, gz), (2, gn)):
                flat = tile_.rearrange("p t b -> p (t b)")
                brow = (sb["brz_row"][:, gate * H:(gate + 1) * H] if gate < 2
                        else sb["bin_row"])
                nc.tensor.matmul(flat, brow, sb["ones_row"][:, 0:NF],
                                 start=True, stop=False, skip_group_check=True)
                for c in range(2):
                    nc.tensor.matmul(flat, sb["wihT"][:, c, gate * H:(gate + 1) * H],
                                     xsb[:, c, t0:t0 + GI_BLK, :].rearrange("p t b -> p (t b)"),
                                     start=False, stop=False, skip_group_check=True)
            gi_banks[gblk] = (gr, gz, gn)

        # ---------- per-step tail: gh, gates, fusion ----------
        out_tiles = {}
        fus_static = singles.tile([H, Bc], f32, tag="fus_static")

        def emit_step_tail(t, whp, whp_sb, dgfA=None):
            """whp: psum (or sbuf) [H, Bc] holding 4*weighted_h; whp_sb: sbuf copy."""
            gblk, ti = t // GI_BLK, t % GI_BLK
            gr, gz, gn = gi_banks[gblk]
            # gh accumulation into gi slices
            nc.tensor.matmul(gr[:, ti, :], sb["whh4T"][:, 0:H], whp_sb,
                             start=False, stop=True, skip_group_check=True)
            nc.tensor.matmul(gz[:, ti, :], sb["whh4T"][:, H:2 * H], whp_sb,
                             start=False, stop=True, skip_group_check=True)
            phn = phn_sl
            nc.tensor.matmul(phn, sb["bhn_row"], sb["ones_row"][:, 0:Bc], start=True, stop=False)
            nc.tensor.matmul(phn, sb["whh4T"][:, 2 * H:3 * H], whp_sb, start=False, stop=True)
            thr = work.tile([H, Bc], f32, tag="thr")
            nc.scalar.activation(thr, gr[:, ti, :], Act.Tanh, bias=0.0, scale=0.5)
            thz = work.tile([H, Bc], f32, tag="thz")
            nc.scalar.activation(thz, gz[:, ti, :], Act.Tanh, bias=0.0, scale=0.5)
            q = work.tile([H, Bc], f32, tag="q")
            nc.vector.scalar_tensor_tensor(out=q, in0=thr, scalar=1.0, in1=phn,
                                           op0=Alu.add, op1=Alu.mult)
            pren = work.tile([H, Bc], f32, tag="pren")
            nc.vector.scalar_tensor_tensor(out=pren, in0=q, scalar=0.5, in1=gn[:, ti, :],
                                           op0=Alu.mult, op1=Alu.add)
            thn = work.tile([H, Bc], f32, tag="thn")
            nc.scalar.activation(thn, pren, Act.Tanh, bias=0.0, scale=1.0)
            dgf = work.tile([H, Bc], f32, tag="dgf")
            if dgfA is not None:
                nc.vector.tensor_tensor(out=dgf, in0=dgfA, in1=thn, op=Alu.subtract)
            else:
                nc.vector.scalar_tensor_tensor(out=dgf, in0=whp, scalar=0.25, in1=thn,
                                               op0=Alu.mult, op1=Alu.subtract)
            e = work.tile([H, Bc], f32, tag="e")
            nc.vector.scalar_tensor_tensor(out=e, in0=thz, scalar=1.0, in1=dgf,
                                           op0=Alu.add, op1=Alu.mult)
            new_h = work.tile([H, Bc], f32, tag="new_h")
            nc.vector.scalar_tensor_tensor(out=new_h, in0=e, scalar=0.5, in1=thn,
                                           op0=Alu.mult, op1=Alu.add)
            return new_h

        def emit_fusion(t, new_h):
            pout = pout_sl
            nc.tensor.matmul(pout, sb["fuswhT"], new_h, start=True, stop=True)
            ob = t // OUT_BLK
            if ob not in out_tiles:
                out_tiles[ob] = out_sb_pool.tile([H, OUT_BLK, Bc], f32, tag="osb", name=f"osb_{ob}")
            nc.vector.tensor_tensor(out=out_tiles[ob][:, t % OUT_BLK, :], in0=pout,
                                     in1=fus_static, op=Alu.add)
            if t % OUT_BLK == OUT_BLK - 1:
                nc.sync.dma_start(out=out_d[:, t - OUT_BLK + 1:t + 1, :], in_=out_tiles[ob])
                del out_tiles[ob]

        # ---------- prologue ----------
        emit_phaseA(0)
        emit_phaseA(1)
        emit_phaseA(2)
        emit_gi_block(0)
        emit_gi_block(1)

        # fus_static = fuswsT@staticT + fusb
        pfs = pfs_sl
        nc.tensor.matmul(pfs, sb["fusb_row"], sb["ones_row"][:, 0:Bc], start=True, stop=False)
        nc.tensor.matmul(pfs, sb["fuswsT"], sb["staticT"], start=False, stop=True)
        nc.scalar.copy(fus_static, pfs)

        # t = 0: wh' = 4*cur0
        pinit = pinit_sl
        nc.tensor.matmul(pinit, sb["initb_row"], sb["ones_row"][:, 0:Bc], start=True, stop=False)
        nc.tensor.matmul(pinit, sb["initwT"], sb["staticT"], start=False, stop=True)
        def emit_maintenance(t):
            """After new_h_t (cur_h): pu1_pre(t+1), rsum update, ring_m write.
            pu1_pre mms carry no new_h dependency, so the close-mm of step
            t+1 is the only PE op on the recurrence cycle."""
            s2 = t % A
            if t + 1 < Tn:
                nc.tensor.matmul(pu1_sl, sb["w1sT"], sb["staticT"], start=True, stop=False)
                nc.tensor.matmul(pu1_sl, sb["w1h10T"], rsum, start=False, stop=False)
            nc.gpsimd.tensor_tensor(out=rsum, in0=rsum, in1=cur_h, op=Alu.add)
            s3 = (t + 1) % A
            if t + 1 < Tn:
                nc.gpsimd.tensor_tensor(out=rsum, in0=rsum, in1=ring_m[:, s3, :],
                                        op=Alu.subtract)
            nc.gpsimd.tensor_copy(ring_m[:, s2, :], cur_h)

        def emit_ring_transpose(t, h_t):
            nc.tensor.matmul(ptr_sl, h_t, sb["ident"],
                             is_transpose=True, start=True, stop=True)
            emit_fusion(t, h_t)

        def emit_ring_copy(t):
            s2 = t % A
            half2, sq2w = s2 % 2, s2 // 2
            if half2 == 0:
                nc.scalar.copy(ring2[0:Bc, sq2w, :], ptr_sl)
            else:
                hT = work.tile([Bc, H], f32, tag="hT")
                nc.scalar.copy(hT, ptr_sl)
                nc.sync.dma_start(out=ring2[Bc:2 * Bc, sq2w, :], in_=hT)

        wh0 = work.tile([H, Bc], f32, tag="wh0")
        nc.vector.tensor_scalar(out=wh0, in0=pinit, scalar1=4.0, scalar2=None, op0=Alu.mult)
        cur_h = emit_step_tail(0, wh0, wh0)
        emit_maintenance(0)

        # ---------- scan t = 1..Tn-1 ----------
        for t in range(1, Tn):
            blk, ti = t // 4, t % 4
            s = (t - 1) % A
            tt = t % 10

            # on-path: close the pu1 accumulation with the h_{t-1} term
            pu1 = pu1_sl
            nc.tensor.matmul(pu1, sb["w1h10T"], cur_h, start=False, stop=True)
            emit_ring_transpose(t - 1, cur_h)
            u1t = work.tile([U, Bc], f32, tag="u1t")
            nc.scalar.activation(u1t, pu1, Act.Tanh, bias=sb["a1b1"], scale=1.0)
            emit_ring_copy(t - 1)

            # logits1 in col-doubled layout [64, 20]; both 10-blocks hold all
            # 10 actions (permuted), so one row-max covers both.
            pl1 = pl1_sl
            for hf in range(2):
                nc.tensor.matmul(pl1[:, hf * A:(hf + 1) * A], sb["ones_row"][:, 0:Bc],
                                 sb["b2re1"][:, tt, hf, :], start=(hf == 0), stop=False)
                nc.tensor.matmul(pl1[:, hf * A:(hf + 1) * A], u1t,
                                 sb["w2re1"][:, tt, hf, :], start=False, stop=(hf == 1))
            rmax1 = work.tile([Bc, 1], f32, tag="rmax1")
            nc.vector.tensor_reduce(out=rmax1, in_=pl1, axis=Axis.X, op=Alu.max)
            cmask = work.tile([Bc, 2 * A], f32, tag="cmask")
            nc.vector.scalar_tensor_tensor(out=cmask, in0=pl1, scalar=rmax1[:, 0:1],
                                           in1=pre2_tiles[blk][:, ti, :],
                                           op0=Alu.is_ge, op1=Alu.add)
            # spread the per-half gather columns to partitions (half, b)
            cmd = cmd_sl
            nc.tensor.matmul(cmd, sb["SPL"], cmask[:, 0:5], start=True, stop=False)
            nc.tensor.matmul(cmd, sb["SPU"], cmask[:, A:A + 5], start=False, stop=True)
            # D[p, (s', b)] = cmd[p, s'] * eye2[p, b], then
            # wh'[h, b] = sum_s' ring2[:, :, s'].T @ D[:, s', :]
            Dm = work.tile([2 * Bc, 5, Bc], f32, tag="Dm")
            cm_b = bass.AP(tensor=cmd.tensor, offset=cmd.offset,
                           ap=[cmd.ap[0], [cmd.ap[1][0], 5], [0, Bc]])
            e2_b = bass.AP(tensor=sb["E2"].tensor, offset=sb["E2"].offset,
                           ap=[sb["E2"].ap[0], [0, 5], [sb["E2"].ap[1][0], Bc]])
            nc.vector.tensor_tensor(out=Dm, in0=cm_b, in1=e2_b, op=Alu.mult)
            pwh = pwh_sl
            for sq2 in range(5):
                nc.tensor.matmul(pwh, ring2[:, sq2, :], Dm[:, sq2, :],
                                 start=(sq2 == 0), stop=(sq2 == 4))
            whp_sb = work.tile([H, Bc], f32, tag="whp_sb")
            nc.vector.tensor_copy(whp_sb, pwh)
            dgfA = work.tile([H, Bc], f32, tag="dgfA")
            nc.gpsimd.tensor_scalar(out=dgfA, in0=whp_sb, scalar1=0.25, scalar2=None,
                                    op0=Alu.mult)

            cur_h = emit_step_tail(t, pwh, whp_sb, dgfA)
            emit_maintenance(t)
            if t == Tn - 1:
                emit_ring_transpose(t, cur_h)
                emit_ring_copy(t)
            # bulk blocks emitted after the chain so they fill PE gaps
            if ti == 1:
                if (blk + 1) * GI_BLK < Tn:
                    emit_gi_block(blk + 1)
                if (blk + 2) * PA_BLK < Tn:
                    emit_phaseA(blk + 2)
                if blk - 2 in pre2_tiles:
                    del pre2_tiles[blk - 2]

    nc.compile()
    _BUILD_CACHE[key] = (nc, "out")
    return _BUILD_CACHE[key]


def _prep_core_inputs(inputs, core, Tn=T, Bc=BC):
    f = np.float32
    b0 = core * Bc
    x = np.ascontiguousarray(inputs["x"][b0:b0 + Bc, :Tn, :]).astype(f)
    xT = np.ascontiguousarray(x.transpose(2, 1, 0).reshape(2, 128, Tn, Bc))
    staticT = np.ascontiguousarray(inputs["static"][b0:b0 + Bc].T).astype(f)
    wih = inputs["gru_wih"].astype(f); whh = inputs["gru_whh"].astype(f)
    a1w1 = inputs["a1_w1"].astype(f); a2w1 = inputs["a2_w1"].astype(f)
    bih = inputs["gru_bih"].astype(f); bhh = inputs["gru_bhh"].astype(f)
    w2_1 = inputs["a1_w2"].astype(f); w2_2 = inputs["a2_w2"].astype(f)
    b2_1 = inputs["a1_b2"].astype(f); b2_2 = inputs["a2_b2"].astype(f)

    # permuted second-layer weights: col j of variant (tt, half) = row a of w2
    # with a = (phys(j, half) - tt) mod 10
    w2re1 = np.zeros((10, 2, U, A), f); w2re2 = np.zeros((10, 2, U, A), f)
    b2re1 = np.zeros((10, 2, A), f); b2re2 = np.zeros((10, 2, A), f)
    for tt in range(10):
        for hf in range(2):
            for j in range(A):
                a = (_phys(j, hf) - tt) % 10
                w2re1[tt, hf, :, j] = w2_1[a, :]
                w2re2[tt, hf, :, j] = w2_2[a, :]
                b2re1[tt, hf, j] = b2_1[a]
                b2re2[tt, hf, j] = b2_2[a]

    # deltas: per phase-A block variant v (t0%10 = 2v cycle {0,8,6,4,2}),
    # 8 step-columns; each step t marks column j where phys(j, half) == (t-1)%10
    # with 2.0 (the 0.5*cur_h direct term, x4 scale)
    deltas = np.zeros((5, Bc, 4 * 2 * A), f)
    for blk_v in range(5):
        t0mod = (blk_v * 4) % 10
        for ti in range(4):
            tmod = (t0mod + ti) % 10
            s9 = (tmod - 1) % 10
            for hf in range(2):
                for j in range(A):
                    if _phys(j, hf) == s9:
                        deltas[blk_v, :, ti * 2 * A + hf * A + j] = 2.0

    m = {
        "xT": xT,
        "staticT": staticT,
        "static_rep": np.tile(staticT, (1, 8)),
        "wihT": np.ascontiguousarray(wih.T.reshape(2, 128, G3)),
        "whh4T": np.ascontiguousarray((whh / 4.0).T),
        "w1h10T": np.ascontiguousarray((a1w1[:, :H] / 10.0).T),
        "w1h10Tn": np.ascontiguousarray((-a1w1[:, :H] / 10.0).T),
        "w1sT": np.ascontiguousarray(a1w1[:, H:].T),
        "a2w1xT": np.ascontiguousarray(a2w1[:, :D].T.reshape(2, 128, U)),
        "a2w1sT": np.ascontiguousarray(a2w1[:, D:].T),
        "w2re1": w2re1, "w2re2": w2re2, "b2re1": b2re1, "b2re2": b2re2,
        "deltas": deltas,
        "initwT": np.ascontiguousarray(inputs["init_w"].astype(f).T),
        "fuswhT": np.ascontiguousarray(inputs["fus_w"].astype(f)[:, :H].T),
        "fuswsT": np.ascontiguousarray(inputs["fus_w"].astype(f)[:, H:].T),
        "a1b1": inputs["a1_b1"].astype(f).reshape(U, 1),
        "a2b1": inputs["a2_b1"].astype(f).reshape(U, 1),
        "brz_row": (bih[:2 * H] + bhh[:2 * H]).reshape(1, 2 * H).copy(),
        "bin_row": bih[2 * H:].reshape(1, H).copy(),
        "bhn_row": bhh[2 * H:].reshape(1, H).copy(),
        "ones_row": np.ones((1, 8 * Bc), f),
        "fusb_row": inputs["fus_b"].astype(f).reshape(1, H).copy(),
        "initb_row": inputs["init_b"].astype(f).reshape(1, H).copy(),
        "E2": np.tile(np.eye(Bc, dtype=f), (2, 1)),
        "SPL": np.concatenate([np.eye(Bc, dtype=f), np.zeros((Bc, Bc), f)], axis=1),
        "SPU": np.concatenate([np.zeros((Bc, Bc), f), np.eye(Bc, dtype=f)], axis=1),
        "ident": np.eye(128, dtype=f),
    }
    return {k: np.ascontiguousarray(v, dtype=f) for k, v in m.items()}


def kernel(**inputs):
    from concourse.bass_utils import run_bass_kernel_spmd
    nc, _ = _build(T, BC)
    in_maps = [_prep_core_inputs(inputs, c) for c in range(NCORES)]
    res = run_bass_kernel_spmd(nc, in_maps, core_ids=list(range(NCORES)))
    out = np.empty((B, T, H), np.float32)
    for c in range(NCORES):
        oc = res.results[c]["out"]
        out[c * BC:(c + 1) * BC] = oc.transpose(2, 1, 0)
    return out


# revision 5
# speedup vs baseline: 1.1619x; 1.0315x over previous
"""Trainium2 Bass kernel v2 for the AgentLayer GRU-with-action-memory model.

Data-parallel over batch: B=512 -> 8 cores x Bc=64. Per-core redesign vs v1:
  - gi (wih@x) computed in bulk 8-step blocks into PSUM banks; per-step gh
    matmuls accumulate into the same bank slices (skip_group_check).
  - argmax masks via is_ge against the row max (gaps verified > 2.7e-7 on
    this seed, so the max is unique); logits computed in a "doubled" layout
    [128 = 2 half-copies of b, 10 cols in per-half physical-slot order].
  - gather = one DVE/Pool masked-multiply over the ring
    [(slot%2)*64+b, h, slot//2] + 5 accumulating PE matmuls against a
    constant tiled-identity E2, producing weighted_h' = 4*weighted_h in
    [H, Bc] PSUM directly.  whh is pre-divided by 4 (exact) to compensate.
  - All matmuls strictly fp32 (fp32r/bf16 flip argmax decisions; verified).
"""

import numpy as np
from contextlib import ExitStack

B, T, D, H, S, A, U = 512, 256, 256, 128, 64, 10, 64
NCORES = 8
BC = B // NCORES  # 64
G3 = 3 * H        # 384

_BUILD_CACHE = {}


def _phys(j, upper):
    if j < 5:
        return 2 * j + (1 if upper else 0)
    return 2 * (j - 5) + (0 if upper else 1)


def _build(Tn, Bc):
    key = (Tn, Bc)
    if key in _BUILD_CACHE:
        return _BUILD_CACHE[key]

    import concourse.bass as bass
    import concourse.bacc as bacc
    import concourse.tile as tile
    from concourse import mybir

    f32 = mybir.dt.float32
    Alu = mybir.AluOpType
    Act = mybir.ActivationFunctionType
    Axis = mybir.AxisListType

    nc = bacc.Bacc("TRN2", target_bir_lowering=False, debug=False)

    d_in = {}

    def din(name, shape):
        d_in[name] = nc.dram_tensor(name, list(shape), f32, kind="ExternalInput").ap()
        return d_in[name]

    xT = din("xT", (2, 128, Tn, Bc))          # x as [chunk, d, t, b]
    staticT = din("staticT", (S, Bc))
    static_rep = din("static_rep", (S, 8 * Bc))
    wihT = din("wihT", (2, 128, G3))
    whh4T = din("whh4T", (H, G3))             # (whh/4).T
    w1h10T = din("w1h10T", (H, U))            # (a1_w1[:, :H]/10).T
    w1h10Tn = din("w1h10Tn", (H, U))          # negated
    w1sT = din("w1sT", (S, U))
    a2w1xT = din("a2w1xT", (2, 128, U))
    a2w1sT = din("a2w1sT", (S, U))
    w2re1 = din("w2re1", (10, 2, U, A))       # a1_w2.T column-permuted per t%10, half
    w2re2 = din("w2re2", (10, 2, U, A))       # a2_w2.T likewise
    b2re1 = din("b2re1", (10, 2, A))          # a1_b2 permuted rows
    b2re2 = din("b2re2", (10, 2, A))
    deltas = din("deltas", (5, Bc, 8 * A))    # phase-A newest-slot marks, col-doubled
    initwT = din("initwT", (S, H))
    fuswhT = din("fuswhT", (H, H))
    fuswsT = din("fuswsT", (S, H))
    a1b1 = din("a1b1", (U, 1))
    a2b1 = din("a2b1", (U, 1))
    brz_row = din("brz_row", (1, 2 * H))      # bih+bhh for r,z gates
    bin_row = din("bin_row", (1, H))          # bih n-gate
    bhn_row = din("bhn_row", (1, H))          # bhh n-gate
    ones_row = din("ones_row", (1, 8 * Bc))
    fusb_row = din("fusb_row", (1, H))
    initb_row = din("initb_row", (1, H))
    E2 = din("E2", (2 * Bc, Bc))              # tiled identity
    SPL = din("SPL", (Bc, 2 * Bc))            # [eye | 0] spread-to-lower
    SPU = din("SPU", (Bc, 2 * Bc))            # [0 | eye] spread-to-upper
    ident = din("ident", (128, 128))

    out_d = nc.dram_tensor("out", [H, Tn, Bc], f32, kind="ExternalOutput").ap()

    NBLK = Tn // 8
    assert Tn % 8 == 0
    OUT_BLK = 16

    with ExitStack() as ctx:
        tc = ctx.enter_context(tile.TileContext(nc))
        singles = ctx.enter_context(tc.tile_pool(name="singles", bufs=1))
        work = ctx.enter_context(tc.tile_pool(name="work", bufs=3))
        psg = ctx.enter_context(tc.tile_pool(name="psg", bufs=2, space="PSUM"))     # gi banks (3/blk)
        psfix = ctx.enter_context(tc.tile_pool(name="psfix", bufs=1, space="PSUM"))  # 2 packed banks
        u2t_pool = ctx.enter_context(tc.tile_pool(name="u2t", bufs=2))
        pre2_pool = ctx.enter_context(tc.tile_pool(name="pre2", bufs=4))
        out_sb_pool = ctx.enter_context(tc.tile_pool(name="outsb", bufs=2))

        # ---- load constants / weights into SBUF ----
        sb = {}
        for name, ap in d_in.items():
            if name == "xT":
                continue
            if name in ("wihT", "a2w1xT"):
                t = singles.tile([128, 2, ap.shape[2]], f32, tag=f"w_{name}", name=f"w_{name}")
                for c in range(2):
                    nc.sync.dma_start(out=t[:, c, :], in_=ap[c])
            elif name in ("w2re1", "w2re2"):
                t = singles.tile([U, 10, 2, A], f32, tag=f"w_{name}", name=f"w_{name}")
                nc.sync.dma_start(out=t, in_=bass.AP(
                    tensor=ap.tensor, offset=ap.offset,
                    ap=[[ap.ap[2][0], U], [ap.ap[0][0], 10], [ap.ap[1][0], 2], [ap.ap[3][0], A]]))
            elif name in ("b2re1", "b2re2"):
                t = singles.tile([1, 10, 2, A], f32, tag=f"w_{name}", name=f"w_{name}")
                nc.sync.dma_start(out=t, in_=bass.AP(
                    tensor=ap.tensor, offset=ap.offset,
                    ap=[[0, 1], [ap.ap[0][0], 10], [ap.ap[1][0], 2], [ap.ap[2][0], A]]))
            elif name == "deltas":
                t = singles.tile([Bc, 5, 8 * A], f32, tag="w_deltas", name="w_deltas")
                nc.sync.dma_start(out=t, in_=bass.AP(
                    tensor=ap.tensor, offset=ap.offset,
                    ap=[[ap.ap[1][0], Bc], [ap.ap[0][0], 5], [ap.ap[2][0], 8 * A]]))
            else:
                t = singles.tile(list(ap.shape), f32, tag=f"w_{name}", name=f"w_{name}")
                nc.sync.dma_start(out=t, in_=ap)
            sb[name] = t

        xsb = singles.tile([128, 2, Tn, Bc], f32, tag="xsb")
        for c in range(2):
            nc.sync.dma_start(out=xsb[:, c, :, :], in_=xT[c])

        # ---- persistent state ----
        ring2 = singles.tile([2 * Bc, 5, H], f32, tag="ring2")   # [(s%2)*64+b, s//2, h]
        ring_m = singles.tile([H, A, Bc], f32, tag="ring_m")     # [h, slot, b]
        rsum = singles.tile([H, Bc], f32, tag="rsum")
        nc.vector.memset(ring2, 0.0)
        nc.vector.memset(ring_m, 0.0)
        nc.vector.memset(rsum, 0.0)

        pre2_tiles = {}

        # two packed psum banks, manually partitioned by column range
        psA_t = psfix.tile([128, 512], f32, tag="psA")
        psB_t = psfix.tile([128, 512], f32, tag="psB")
        pu2_sl = psA_t[0:U, 0:256]
        pl2_sl = psA_t[0:Bc, 256:336].rearrange("p (t a) -> p t a", a=2 * A)
        ptr_sl = psA_t[0:Bc, 336:464]
        pu1_sl = psB_t[0:U, 0:Bc]
        pl1_sl = psB_t[0:Bc, 64:64 + 2 * A]
        cmd_sl = psB_t[:, 88:93]
        pwh_sl = psB_t[:, 128:192]
        phn_sl = psB_t[:, 192:256]
        pout_sl = psB_t[:, 256:320]
        pfs_sl = psB_t[:, 320:384]
        pinit_sl = psB_t[:, 384:448]

        # ---------- phase A: pre2_d for a 4-step block ----------
        PA_BLK = 4
        def emit_phaseA(blk):
            t0 = blk * PA_BLK
            NF = PA_BLK * Bc
            pu2 = pu2_sl
            for c in range(2):
                nc.tensor.matmul(pu2, sb["a2w1xT"][:, c, :],
                                 xsb[:, c, t0:t0 + PA_BLK, :].rearrange("p t b -> p (t b)"),
                                 start=(c == 0), stop=False)
            nc.tensor.matmul(pu2, sb["a2w1sT"], sb["static_rep"][:, 0:NF],
                             start=False, stop=True)
            u2t = u2t_pool.tile([U, NF], f32, tag="u2t")
            nc.scalar.activation(u2t, pu2, Act.Tanh, bias=sb["a2b1"], scale=1.0)
            pl2 = pl2_sl
            for ti in range(PA_BLK):
                tt = (t0 + ti) % 10
                for half in range(2):
                    nc.tensor.matmul(pl2[:, ti, half * A:(half + 1) * A],
                                     sb["ones_row"][:, 0:Bc], sb["b2re2"][:, tt, half, :],
                                     start=(half == 0), stop=False)
                    nc.tensor.matmul(pl2[:, ti, half * A:(half + 1) * A],
                                     u2t[:, ti * Bc:(ti + 1) * Bc],
                                     sb["w2re2"][:, tt, half, :],
                                     start=False, stop=(half == 1))
            rmax2 = work.tile([Bc, PA_BLK], f32, tag="rmax2")
            nc.vector.tensor_reduce(out=rmax2, in_=pl2, axis=Axis.X, op=Alu.max)
            rmax2_b = bass.AP(tensor=rmax2.tensor, offset=rmax2.offset,
                              ap=[rmax2.ap[0], rmax2.ap[1], [0, 2 * A]])
            ge2 = work.tile([Bc, PA_BLK, 2 * A], f32, tag="ge2")
            nc.vector.tensor_tensor(out=ge2, in0=pl2, in1=rmax2_b, op=Alu.is_ge)
            pre2 = pre2_pool.tile([Bc, PA_BLK, 2 * A], f32, tag="pre2", name=f"pre2_{blk}")
            nc.gpsimd.tensor_tensor(
                out=pre2, in0=ge2,
                in1=sb["deltas"][:, blk % 5, :].rearrange("p (t a) -> p t a", a=2 * A),
                op=Alu.add)
            pre2_tiles[blk] = pre2

        # ---------- bulk gi for a 4-step block (GI_BLK=4 keeps each psum
        # tile within one 2KB bank) ----------
        GI_BLK = 4
        gi_banks = {}

        def emit_gi_block(gblk):
            t0 = gblk * GI_BLK
            NF = GI_BLK * Bc
            gr = psg.tile([128, GI_BLK, Bc], f32, tag="gir", name=f"gir_{gblk}")
            gz = psg.tile([128, GI_BLK, Bc], f32, tag="giz", name=f"giz_{gblk}")
            gn = psg.tile([128, GI_BLK, Bc], f32, tag="gin", name=f"gin_{gblk}")
            for gate, tile_ in ((0, gr), (1, gz), (2, gn)):
                flat = tile_.rearrange("p t b -> p (t b)")
                brow = (sb["brz_row"][:, gate * H:(gate + 1) * H] if gate < 2
                        else sb["bin_row"])
                nc.tensor.matmul(flat, brow, sb["ones_row"][:, 0:NF],
                                 start=True, stop=False, skip_group_check=True)
                for c in range(2):
                    nc.tensor.matmul(flat, sb["wihT"][:, c, gate * H:(gate + 1) * H],
                                     xsb[:, c, t0:t0 + GI_BLK, :].rearrange("p t b -> p (t b)"),
                                     start=False, stop=False, skip_group_check=True)
            gi_banks[gblk] = (gr, gz, gn)

        # ---------- per-step tail: gh, gates, fusion ----------
        out_tiles = {}
        fus_static = singles.tile([H, Bc], f32, tag="fus_static")

        def emit_step_tail(t, whp, whp_sb, dgfA=None):
            """whp: psum (or sbuf) [H, Bc] holding 4*weighted_h; whp_sb: sbuf copy."""
            gblk, ti = t // GI_BLK, t % GI_BLK
            gr, gz, gn = gi_banks[gblk]
            # gh accumulation into gi slices
            nc.tensor.matmul(gr[:, ti, :], sb["whh4T"][:, 0:H], whp_sb,
                             start=False, stop=True, skip_group_check=True)
            phn = phn_sl
            nc.tensor.matmul(phn, sb["bhn_row"], sb["ones_row"][:, 0:Bc], start=True, stop=False)
            nc.tensor.matmul(phn, sb["whh4T"][:, 2 * H:3 * H], whp_sb, start=False, stop=True)
            nc.tensor.matmul(gz[:, ti, :], sb["whh4T"][:, H:2 * H], whp_sb,
                             start=False, stop=True, skip_group_check=True)
            thr = work.tile([H, Bc], f32, tag="thr")
            nc.scalar.activation(thr, gr[:, ti, :], Act.Tanh, bias=0.0, scale=0.5)
            thz = work.tile([H, Bc], f32, tag="thz")
            nc.scalar.activation(thz, gz[:, ti, :], Act.Tanh, bias=0.0, scale=0.5)
            q = work.tile([H, Bc], f32, tag="q")
            nc.vector.scalar_tensor_tensor(out=q, in0=thr, scalar=1.0, in1=phn,
                                           op0=Alu.add, op1=Alu.mult)
            pren = work.tile([H, Bc], f32, tag="pren")
            nc.vector.scalar_tensor_tensor(out=pren, in0=q, scalar=0.5, in1=gn[:, ti, :],
                                           op0=Alu.mult, op1=Alu.add)
            # new_h = 0.5(1+thz)*dgfA' + 0.5(1-thz)*thn  with dgfA' = 0.25wh'
            # (dgfA here = 0.125*wh' so A1 = 0.5(1+thz)*0.25wh').  A1/B1 need
            # only thz, so they run while thn's tanh is still in flight.
            if dgfA is not None:
                A1 = work.tile([H, Bc], f32, tag="A1")
                nc.vector.scalar_tensor_tensor(out=A1, in0=thz, scalar=1.0, in1=dgfA,
                                               op0=Alu.add, op1=Alu.mult)
                B1 = work.tile([H, Bc], f32, tag="B1")
                nc.gpsimd.tensor_scalar(out=B1, in0=thz, scalar1=-0.5, scalar2=0.5,
                                        op0=Alu.mult, op1=Alu.add)
            thn = work.tile([H, Bc], f32, tag="thn")
            nc.scalar.activation(thn, pren, Act.Tanh, bias=0.0, scale=1.0)
            new_h = work.tile([H, Bc], f32, tag="new_h")
            if dgfA is not None:
                tmp = work.tile([H, Bc], f32, tag="nhtmp")
                nc.vector.tensor_tensor(out=tmp, in0=B1, in1=thn, op=Alu.mult)
                nc.vector.tensor_tensor(out=new_h, in0=tmp, in1=A1, op=Alu.add)
            else:
                dgf = work.tile([H, Bc], f32, tag="dgf")
                nc.vector.scalar_tensor_tensor(out=dgf, in0=whp, scalar=0.25, in1=thn,
                                               op0=Alu.mult, op1=Alu.subtract)
                e = work.tile([H, Bc], f32, tag="e")
                nc.vector.scalar_tensor_tensor(out=e, in0=thz, scalar=1.0, in1=dgf,
                                               op0=Alu.add, op1=Alu.mult)
                nc.vector.scalar_tensor_tensor(out=new_h, in0=e, scalar=0.5, in1=thn,
                                               op0=Alu.mult, op1=Alu.add)
            return new_h

        def emit_fusion(t, new_h):
            pout = pout_sl
            nc.tensor.matmul(pout, sb["fuswhT"], new_h, start=True, stop=True)
            ob = t // OUT_BLK
            if ob not in out_tiles:
                out_tiles[ob] = out_sb_pool.tile([H, OUT_BLK, Bc], f32, tag="osb", name=f"osb_{ob}")
            nc.vector.tensor_tensor(out=out_tiles[ob][:, t % OUT_BLK, :], in0=pout,
                                     in1=fus_static, op=Alu.add)
            if t % OUT_BLK == OUT_BLK - 1:
                nc.sync.dma_start(out=out_d[:, t - OUT_BLK + 1:t + 1, :], in_=out_tiles[ob])
                del out_tiles[ob]

        # ---------- prologue ----------
        emit_phaseA(0)
        emit_phaseA(1)
        emit_phaseA(2)
        emit_gi_block(0)
        emit_gi_block(1)

        # fus_static = fuswsT@staticT + fusb
        pfs = pfs_sl
        nc.tensor.matmul(pfs, sb["fusb_row"], sb["ones_row"][:, 0:Bc], start=True, stop=False)
        nc.tensor.matmul(pfs, sb["fuswsT"], sb["staticT"], start=False, stop=True)
        nc.scalar.copy(fus_static, pfs)

        # t = 0: wh' = 4*cur0
        pinit = pinit_sl
        nc.tensor.matmul(pinit, sb["initb_row"], sb["ones_row"][:, 0:Bc], start=True, stop=False)
        nc.tensor.matmul(pinit, sb["initwT"], sb["staticT"], start=False, stop=True)
        def emit_maintenance(t):
            """After new_h_t (cur_h): pu1_pre(t+1), rsum update, ring_m write.
            pu1_pre mms carry no new_h dependency, so the close-mm of step
            t+1 is the only PE op on the recurrence cycle."""
            s2 = t % A
            if t + 1 < Tn:
                nc.tensor.matmul(pu1_sl, sb["w1sT"], sb["staticT"], start=True, stop=False)
                nc.tensor.matmul(pu1_sl, sb["w1h10T"], rsum, start=False, stop=False)
            nc.gpsimd.tensor_tensor(out=rsum, in0=rsum, in1=cur_h, op=Alu.add)
            s3 = (t + 1) % A
            if t + 1 < Tn:
                nc.gpsimd.tensor_tensor(out=rsum, in0=rsum, in1=ring_m[:, s3, :],
                                        op=Alu.subtract)
            nc.gpsimd.tensor_copy(ring_m[:, s2, :], cur_h)

        def emit_ring_transpose(t, h_t):
            nc.tensor.matmul(ptr_sl, h_t, sb["ident"],
                             is_transpose=True, start=True, stop=True)
            emit_fusion(t, h_t)

        def emit_ring_copy(t):
            s2 = t % A
            half2, sq2w = s2 % 2, s2 // 2
            if half2 == 0:
                nc.scalar.copy(ring2[0:Bc, sq2w, :], ptr_sl)
            else:
                hT = work.tile([Bc, H], f32, tag="hT")
                nc.scalar.copy(hT, ptr_sl)
                nc.sync.dma_start(out=ring2[Bc:2 * Bc, sq2w, :], in_=hT)

        wh0 = work.tile([H, Bc], f32, tag="wh0")
        nc.vector.tensor_scalar(out=wh0, in0=pinit, scalar1=4.0, scalar2=None, op0=Alu.mult)
        cur_h = emit_step_tail(0, wh0, wh0)
        emit_maintenance(0)

        # ---------- scan t = 1..Tn-1 ----------
        for t in range(1, Tn):
            blk, ti = t // 4, t % 4
            s = (t - 1) % A
            tt = t % 10

            # on-path: close the pu1 accumulation with the h_{t-1} term
            pu1 = pu1_sl
            nc.tensor.matmul(pu1, sb["w1h10T"], cur_h, start=False, stop=True)
            emit_ring_transpose(t - 1, cur_h)
            u1t = work.tile([U, Bc], f32, tag="u1t")
            nc.scalar.activation(u1t, pu1, Act.Tanh, bias=sb["a1b1"], scale=1.0)
            emit_ring_copy(t - 1)

            # logits1 in col-doubled layout [64, 20]; both 10-blocks hold all
            # 10 actions (permuted), so one row-max covers both.
            pl1 = pl1_sl
            for hf in range(2):
                nc.tensor.matmul(pl1[:, hf * A:(hf + 1) * A], sb["ones_row"][:, 0:Bc],
                                 sb["b2re1"][:, tt, hf, :], start=(hf == 0), stop=False)
                nc.tensor.matmul(pl1[:, hf * A:(hf + 1) * A], u1t,
                                 sb["w2re1"][:, tt, hf, :], start=False, stop=(hf == 1))
            rmax1 = work.tile([Bc, 1], f32, tag="rmax1")
            nc.vector.tensor_reduce(out=rmax1, in_=pl1, axis=Axis.X, op=Alu.max)
            cmask = work.tile([Bc, 2 * A], f32, tag="cmask")
            nc.vector.scalar_tensor_tensor(out=cmask, in0=pl1, scalar=rmax1[:, 0:1],
                                           in1=pre2_tiles[blk][:, ti, :],
                                           op0=Alu.is_ge, op1=Alu.add)
            # spread the per-half gather columns to partitions (half, b)
            cmd = cmd_sl
            nc.tensor.matmul(cmd, sb["SPL"], cmask[:, 0:5], start=True, stop=False)
            nc.tensor.matmul(cmd, sb["SPU"], cmask[:, A:A + 5], start=False, stop=True)
            # D[p, (s', b)] = cmd[p, s'] * eye2[p, b], then
            # wh'[h, b] = sum_s' ring2[:, :, s'].T @ D[:, s', :]
            Dm = work.tile([2 * Bc, 5, Bc], f32, tag="Dm")
            cm_b = bass.AP(tensor=cmd.tensor, offset=cmd.offset,
                           ap=[cmd.ap[0], [cmd.ap[1][0], 5], [0, Bc]])
            e2_b = bass.AP(tensor=sb["E2"].tensor, offset=sb["E2"].offset,
                           ap=[sb["E2"].ap[0], [0, 5], [sb["E2"].ap[1][0], Bc]])
            nc.vector.tensor_tensor(out=Dm, in0=cm_b, in1=e2_b, op=Alu.mult)
            pwh = pwh_sl
            for sq2 in range(5):
                nc.tensor.matmul(pwh, ring2[:, sq2, :], Dm[:, sq2, :],
                                 start=(sq2 == 0), stop=(sq2 == 4))
            whp_sb = work.tile([H, Bc], f32, tag="whp_sb")
            nc.vector.tensor_copy(whp_sb, pwh)
            dgfA = work.tile([H, Bc], f32, tag="dgfA")
            nc.gpsimd.tensor_scalar(out=dgfA, in0=whp_sb, scalar1=0.125, scalar2=None,
                                    op0=Alu.mult)

            cur_h = emit_step_tail(t, pwh, whp_sb, dgfA)
            emit_maintenance(t)
            if t == Tn - 1:
                emit_ring_transpose(t, cur_h)
                emit_ring_copy(t)
            # bulk blocks emitted after the chain so they fill PE gaps
            if ti == 1:
                if (blk + 1) * GI_BLK < Tn:
                    emit_gi_block(blk + 1)
                if (blk + 2) * PA_BLK < Tn:
                    emit_phaseA(blk + 2)
                if blk - 2 in pre2_tiles:
                    del pre2_tiles[blk - 2]

    nc.compile()
    _BUILD_CACHE[key] = (nc, "out")
    return _BUILD_CACHE[key]


def _prep_core_inputs(inputs, core, Tn=T, Bc=BC):
    f = np.float32
    b0 = core * Bc
    x = np.ascontiguousarray(inputs["x"][b0:b0 + Bc, :Tn, :]).astype(f)
    xT = np.ascontiguousarray(x.transpose(2, 1, 0).reshape(2, 128, Tn, Bc))
    staticT = np.ascontiguousarray(inputs["static"][b0:b0 + Bc].T).astype(f)
    wih = inputs["gru_wih"].astype(f); whh = inputs["gru_whh"].astype(f)
    a1w1 = inputs["a1_w1"].astype(f); a2w1 = inputs["a2_w1"].astype(f)
    bih = inputs["gru_bih"].astype(f); bhh = inputs["gru_bhh"].astype(f)
    w2_1 = inputs["a1_w2"].astype(f); w2_2 = inputs["a2_w2"].astype(f)
    b2_1 = inputs["a1_b2"].astype(f); b2_2 = inputs["a2_b2"].astype(f)

    # permuted second-layer weights: col j of variant (tt, half) = row a of w2
    # with a = (phys(j, half) - tt) mod 10
    w2re1 = np.zeros((10, 2, U, A), f); w2re2 = np.zeros((10, 2, U, A), f)
    b2re1 = np.zeros((10, 2, A), f); b2re2 = np.zeros((10, 2, A), f)
    for tt in range(10):
        for hf in range(2):
            for j in range(A):
                a = (_phys(j, hf) - tt) % 10
                w2re1[tt, hf, :, j] = w2_1[a, :]
                w2re2[tt, hf, :, j] = w2_2[a, :]
                b2re1[tt, hf, j] = b2_1[a]
                b2re2[tt, hf, j] = b2_2[a]

    # deltas: per phase-A block variant v (t0%10 = 2v cycle {0,8,6,4,2}),
    # 8 step-columns; each step t marks column j where phys(j, half) == (t-1)%10
    # with 2.0 (the 0.5*cur_h direct term, x4 scale)
    deltas = np.zeros((5, Bc, 4 * 2 * A), f)
    for blk_v in range(5):
        t0mod = (blk_v * 4) % 10
        for ti in range(4):
            tmod = (t0mod + ti) % 10
            s9 = (tmod - 1) % 10
            for hf in range(2):
                for j in range(A):
                    if _phys(j, hf) == s9:
                        deltas[blk_v, :, ti * 2 * A + hf * A + j] = 2.0

    m = {
        "xT": xT,
        "staticT": staticT,
        "static_rep": np.tile(staticT, (1, 8)),
        "wihT": np.ascontiguousarray(wih.T.reshape(2, 128, G3)),
        "whh4T": np.ascontiguousarray((whh / 4.0).T),
        "w1h10T": np.ascontiguousarray((a1w1[:, :H] / 10.0).T),
        "w1h10Tn": np.ascontiguousarray((-a1w1[:, :H] / 10.0).T),
        "w1sT": np.ascontiguousarray(a1w1[:, H:].T),
        "a2w1xT": np.ascontiguousarray(a2w1[:, :D].T.reshape(2, 128, U)),
        "a2w1sT": np.ascontiguousarray(a2w1[:, D:].T),
        "w2re1": w2re1, "w2re2": w2re2, "b2re1": b2re1, "b2re2": b2re2,
        "deltas": deltas,
        "initwT": np.ascontiguousarray(inputs["init_w"].astype(f).T),
        "fuswhT": np.ascontiguousarray(inputs["fus_w"].astype(f)[:, :H].T),
        "fuswsT": np.ascontiguousarray(inputs["fus_w"].astype(f)[:, H:].T),
        "a1b1": inputs["a1_b1"].astype(f).reshape(U, 1),
        "a2b1": inputs["a2_b1"].astype(f).reshape(U, 1),
        "brz_row": (bih[:2 * H] + bhh[:2 * H]).reshape(1, 2 * H).copy(),
        "bin_row": bih[2 * H:].reshape(1, H).copy(),
        "bhn_row": bhh[2 * H:].reshape(1, H).copy(),
        "ones_row": np.ones((1, 8 * Bc), f),
        "fusb_row": inputs["fus_b"].astype(f).reshape(1, H).copy(),
        "initb_row": inputs["init_b"].astype(f).reshape(1, H).copy(),
        "E2": np.tile(np.eye(Bc, dtype=f), (2, 1)),
        "SPL": np.concatenate([np.eye(Bc, dtype=f), np.zeros((Bc, Bc), f)], axis=1),
        "SPU": np.concatenate([np.zeros((Bc, Bc), f), np.eye(Bc, dtype=f)], axis=1),
        "ident": np.eye(128, dtype=f),
    }
    return {k: np.ascontiguousarray(v, dtype=f) for k, v in m.items()}


def kernel(**inputs):
    from concourse.bass_utils import run_bass_kernel_spmd
    nc, _ = _build(T, BC)
    in_maps = [_prep_core_inputs(inputs, c) for c in range(NCORES)]
    res = run_bass_kernel_spmd(nc, in_maps, core_ids=list(range(NCORES)))
    out = np.empty((B, T, H), np.float32)
    for c in range(NCORES):
        oc = res.results[c]["out"]
        out[c * BC:(c + 1) * BC] = oc.transpose(2, 1, 0)
    return out


# revision 6
# speedup vs baseline: 1.1632x; 1.0012x over previous
"""Trainium2 Bass kernel v2 for the AgentLayer GRU-with-action-memory model.

Data-parallel over batch: B=512 -> 8 cores x Bc=64. Per-core redesign vs v1:
  - gi (wih@x) computed in bulk 8-step blocks into PSUM banks; per-step gh
    matmuls accumulate into the same bank slices (skip_group_check).
  - argmax masks via is_ge against the row max (gaps verified > 2.7e-7 on
    this seed, so the max is unique); logits computed in a "doubled" layout
    [128 = 2 half-copies of b, 10 cols in per-half physical-slot order].
  - gather = one DVE/Pool masked-multiply over the ring
    [(slot%2)*64+b, h, slot//2] + 5 accumulating PE matmuls against a
    constant tiled-identity E2, producing weighted_h' = 4*weighted_h in
    [H, Bc] PSUM directly.  whh is pre-divided by 4 (exact) to compensate.
  - All matmuls strictly fp32 (fp32r/bf16 flip argmax decisions; verified).
"""

import numpy as np
from contextlib import ExitStack

B, T, D, H, S, A, U = 512, 256, 256, 128, 64, 10, 64
NCORES = 8
BC = B // NCORES  # 64
G3 = 3 * H        # 384

_BUILD_CACHE = {}


def _phys(j, upper):
    if j < 5:
        return 2 * j + (1 if upper else 0)
    return 2 * (j - 5) + (0 if upper else 1)


def _build(Tn, Bc):
    key = (Tn, Bc)
    if key in _BUILD_CACHE:
        return _BUILD_CACHE[key]

    import concourse.bass as bass
    import concourse.bacc as bacc
    import concourse.tile as tile
    from concourse import mybir

    f32 = mybir.dt.float32
    Alu = mybir.AluOpType
    Act = mybir.ActivationFunctionType
    Axis = mybir.AxisListType

    nc = bacc.Bacc("TRN2", target_bir_lowering=False, debug=False)

    d_in = {}

    def din(name, shape):
        d_in[name] = nc.dram_tensor(name, list(shape), f32, kind="ExternalInput").ap()
        return d_in[name]

    xT = din("xT", (2, 128, Tn, Bc))          # x as [chunk, d, t, b]
    staticT = din("staticT", (S, Bc))
    static_rep = din("static_rep", (S, 8 * Bc))
    wihT = din("wihT", (2, 128, G3))
    whh4T = din("whh4T", (H, G3))             # (whh/4).T
    w1h10T = din("w1h10T", (H, U))            # (a1_w1[:, :H]/10).T
    w1h10Tn = din("w1h10Tn", (H, U))          # negated
    w1sT = din("w1sT", (S, U))
    a2w1xT = din("a2w1xT", (2, 128, U))
    a2w1sT = din("a2w1sT", (S, U))
    w2re1 = din("w2re1", (10, 2, U, A))       # a1_w2.T column-permuted per t%10, half
    w2re2 = din("w2re2", (10, 2, U, A))       # a2_w2.T likewise
    b2re1 = din("b2re1", (10, 2, A))          # a1_b2 permuted rows
    b2re2 = din("b2re2", (10, 2, A))
    deltas = din("deltas", (5, Bc, 8 * A))    # phase-A newest-slot marks, col-doubled
    initwT = din("initwT", (S, H))
    fuswhT = din("fuswhT", (H, H))
    fuswsT = din("fuswsT", (S, H))
    a1b1 = din("a1b1", (U, 1))
    a2b1 = din("a2b1", (U, 1))
    brz_row = din("brz_row", (1, 2 * H))      # bih+bhh for r,z gates
    bin_row = din("bin_row", (1, H))          # bih n-gate
    bhn_row = din("bhn_row", (1, H))          # bhh n-gate
    ones_row = din("ones_row", (1, 8 * Bc))
    fusb_row = din("fusb_row", (1, H))
    initb_row = din("initb_row", (1, H))
    E2 = din("E2", (2 * Bc, Bc))              # tiled identity
    SPL = din("SPL", (Bc, 2 * Bc))            # [eye | 0] spread-to-lower
    SPU = din("SPU", (Bc, 2 * Bc))            # [0 | eye] spread-to-upper
    ident = din("ident", (128, 128))

    out_d = nc.dram_tensor("out", [H, Tn, Bc], f32, kind="ExternalOutput").ap()

    NBLK = Tn // 8
    assert Tn % 8 == 0
    OUT_BLK = 16

    with ExitStack() as ctx:
        tc = ctx.enter_context(tile.TileContext(nc))
        singles = ctx.enter_context(tc.tile_pool(name="singles", bufs=1))
        work = ctx.enter_context(tc.tile_pool(name="work", bufs=3))
        psg = ctx.enter_context(tc.tile_pool(name="psg", bufs=2, space="PSUM"))     # gi banks (3/blk)
        psfix = ctx.enter_context(tc.tile_pool(name="psfix", bufs=1, space="PSUM"))  # 2 packed banks
        u2t_pool = ctx.enter_context(tc.tile_pool(name="u2t", bufs=2))
        pre2_pool = ctx.enter_context(tc.tile_pool(name="pre2", bufs=4))
        out_sb_pool = ctx.enter_context(tc.tile_pool(name="outsb", bufs=2))

        # ---- load constants / weights into SBUF ----
        sb = {}
        for name, ap in d_in.items():
            if name == "xT":
                continue
            if name in ("wihT", "a2w1xT"):
                t = singles.tile([128, 2, ap.shape[2]], f32, tag=f"w_{name}", name=f"w_{name}")
                for c in range(2):
                    nc.sync.dma_start(out=t[:, c, :], in_=ap[c])
            elif name in ("w2re1", "w2re2"):
                t = singles.tile([U, 10, 2, A], f32, tag=f"w_{name}", name=f"w_{name}")
                nc.sync.dma_start(out=t, in_=bass.AP(
                    tensor=ap.tensor, offset=ap.offset,
                    ap=[[ap.ap[2][0], U], [ap.ap[0][0], 10], [ap.ap[1][0], 2], [ap.ap[3][0], A]]))
            elif name in ("b2re1", "b2re2"):
                t = singles.tile([1, 10, 2, A], f32, tag=f"w_{name}", name=f"w_{name}")
                nc.sync.dma_start(out=t, in_=bass.AP(
                    tensor=ap.tensor, offset=ap.offset,
                    ap=[[0, 1], [ap.ap[0][0], 10], [ap.ap[1][0], 2], [ap.ap[2][0], A]]))
            elif name == "deltas":
                t = singles.tile([Bc, 5, 8 * A], f32, tag="w_deltas", name="w_deltas")
                nc.sync.dma_start(out=t, in_=bass.AP(
                    tensor=ap.tensor, offset=ap.offset,
                    ap=[[ap.ap[1][0], Bc], [ap.ap[0][0], 5], [ap.ap[2][0], 8 * A]]))
            else:
                t = singles.tile(list(ap.shape), f32, tag=f"w_{name}", name=f"w_{name}")
                nc.sync.dma_start(out=t, in_=ap)
            sb[name] = t

        xsb = singles.tile([128, 2, Tn, Bc], f32, tag="xsb")
        for c in range(2):
            nc.sync.dma_start(out=xsb[:, c, :, :], in_=xT[c])

        # ---- persistent state ----
        ring2 = singles.tile([2 * Bc, 5, H], f32, tag="ring2")   # [(s%2)*64+b, s//2, h]
        ring_m = singles.tile([H, A, Bc], f32, tag="ring_m")     # [h, slot, b]
        rsum = singles.tile([H, Bc], f32, tag="rsum")
        nc.vector.memset(ring2, 0.0)
        nc.vector.memset(ring_m, 0.0)
        nc.vector.memset(rsum, 0.0)

        pre2_tiles = {}

        # two packed psum banks, manually partitioned by column range
        psA_t = psfix.tile([128, 512], f32, tag="psA")
        psB_t = psfix.tile([128, 512], f32, tag="psB")
        pu2_sl = psA_t[0:U, 0:256]
        pl2_sl = psA_t[0:Bc, 256:336].rearrange("p (t a) -> p t a", a=2 * A)
        ptr_sl = psA_t[0:Bc, 336:464]
        pu1_sl = psB_t[0:U, 0:Bc]
        pl1_sl = psB_t[0:Bc, 64:64 + 2 * A]
        cmd_sl = psB_t[:, 88:93]
        pwh_sl = psB_t[:, 128:192]
        phn_sl = psB_t[:, 192:256]
        pout_sl = psB_t[:, 256:320]
        pfs_sl = psB_t[:, 320:384]
        pinit_sl = psB_t[:, 384:448]

        # ---------- phase A: pre2_d for a 4-step block ----------
        PA_BLK = 4
        def emit_phaseA(blk):
            t0 = blk * PA_BLK
            NF = PA_BLK * Bc
            pu2 = pu2_sl
            for c in range(2):
                nc.tensor.matmul(pu2, sb["a2w1xT"][:, c, :],
                                 xsb[:, c, t0:t0 + PA_BLK, :].rearrange("p t b -> p (t b)"),
                                 start=(c == 0), stop=False)
            nc.tensor.matmul(pu2, sb["a2w1sT"], sb["static_rep"][:, 0:NF],
                             start=False, stop=True)
            u2t = u2t_pool.tile([U, NF], f32, tag="u2t")
            nc.scalar.activation(u2t, pu2, Act.Tanh, bias=sb["a2b1"], scale=1.0)
            pl2 = pl2_sl
            for ti in range(PA_BLK):
                tt = (t0 + ti) % 10
                for half in range(2):
                    nc.tensor.matmul(pl2[:, ti, half * A:(half + 1) * A],
                                     sb["ones_row"][:, 0:Bc], sb["b2re2"][:, tt, half, :],
                                     start=(half == 0), stop=False)
                    nc.tensor.matmul(pl2[:, ti, half * A:(half + 1) * A],
                                     u2t[:, ti * Bc:(ti + 1) * Bc],
                                     sb["w2re2"][:, tt, half, :],
                                     start=False, stop=(half == 1))
            rmax2 = work.tile([Bc, PA_BLK], f32, tag="rmax2")
            nc.vector.tensor_reduce(out=rmax2, in_=pl2, axis=Axis.X, op=Alu.max)
            rmax2_b = bass.AP(tensor=rmax2.tensor, offset=rmax2.offset,
                              ap=[rmax2.ap[0], rmax2.ap[1], [0, 2 * A]])
            ge2 = work.tile([Bc, PA_BLK, 2 * A], f32, tag="ge2")
            nc.vector.tensor_tensor(out=ge2, in0=pl2, in1=rmax2_b, op=Alu.is_ge)
            pre2 = pre2_pool.tile([Bc, PA_BLK, 2 * A], f32, tag="pre2", name=f"pre2_{blk}")
            nc.gpsimd.tensor_tensor(
                out=pre2, in0=ge2,
                in1=sb["deltas"][:, blk % 5, :].rearrange("p (t a) -> p t a", a=2 * A),
                op=Alu.add)
            pre2_tiles[blk] = pre2

        # ---------- bulk gi for a 4-step block (GI_BLK=4 keeps each psum
        # tile within one 2KB bank) ----------
        GI_BLK = 4
        gi_banks = {}

        def emit_gi_block(gblk):
            t0 = gblk * GI_BLK
            NF = GI_BLK * Bc
            gr = psg.tile([128, GI_BLK, Bc], f32, tag="gir", name=f"gir_{gblk}")
            gz = psg.tile([128, GI_BLK, Bc], f32, tag="giz", name=f"giz_{gblk}")
            gn = psg.tile([128, GI_BLK, Bc], f32, tag="gin", name=f"gin_{gblk}")
            for gate, tile_ in ((0, gr), (1, gz), (2, gn)):
                flat = tile_.rearrange("p t b -> p (t b)")
                brow = (sb["brz_row"][:, gate * H:(gate + 1) * H] if gate < 2
                        else sb["bin_row"])
                nc.tensor.matmul(flat, brow, sb["ones_row"][:, 0:NF],
                                 start=True, stop=False, skip_group_check=True)
                for c in range(2):
                    nc.tensor.matmul(flat, sb["wihT"][:, c, gate * H:(gate + 1) * H],
                                     xsb[:, c, t0:t0 + GI_BLK, :].rearrange("p t b -> p (t b)"),
                                     start=False, stop=False, skip_group_check=True)
            gi_banks[gblk] = (gr, gz, gn)

        # ---------- per-step tail: gh, gates, fusion ----------
        out_tiles = {}
        fus_static = singles.tile([H, Bc], f32, tag="fus_static")

        def emit_step_tail(t, whp, whp_sb, dgfA=None):
            """whp: psum (or sbuf) [H, Bc] holding 4*weighted_h; whp_sb: sbuf copy."""
            gblk, ti = t // GI_BLK, t % GI_BLK
            gr, gz, gn = gi_banks[gblk]
            # gh accumulation into gi slices
            nc.tensor.matmul(gr[:, ti, :], sb["whh4T"][:, 0:H], whp_sb,
                             start=False, stop=True, skip_group_check=True)
            phn = phn_sl
            nc.tensor.matmul(phn, sb["bhn_row"], sb["ones_row"][:, 0:Bc], start=True, stop=False)
            nc.tensor.matmul(phn, sb["whh4T"][:, 2 * H:3 * H], whp_sb, start=False, stop=True)
            nc.tensor.matmul(gz[:, ti, :], sb["whh4T"][:, H:2 * H], whp_sb,
                             start=False, stop=True, skip_group_check=True)
            thr = work.tile([H, Bc], f32, tag="thr")
            nc.scalar.activation(thr, gr[:, ti, :], Act.Tanh, bias=0.0, scale=0.5)
            thz = work.tile([H, Bc], f32, tag="thz")
            nc.scalar.activation(thz, gz[:, ti, :], Act.Tanh, bias=0.0, scale=0.5)
            q = work.tile([H, Bc], f32, tag="q")
            nc.vector.scalar_tensor_tensor(out=q, in0=thr, scalar=1.0, in1=phn,
                                           op0=Alu.add, op1=Alu.mult)
            pren = work.tile([H, Bc], f32, tag="pren")
            nc.vector.scalar_tensor_tensor(out=pren, in0=q, scalar=0.5, in1=gn[:, ti, :],
                                           op0=Alu.mult, op1=Alu.add)
            # new_h = 0.5(1+thz)*dgfA' + 0.5(1-thz)*thn  with dgfA' = 0.25wh'
            # (dgfA here = 0.125*wh' so A1 = 0.5(1+thz)*0.25wh').  A1/B1 need
            # only thz, so they run while thn's tanh is still in flight.
            if dgfA is not None:
                A1 = work.tile([H, Bc], f32, tag="A1")
                nc.vector.scalar_tensor_tensor(out=A1, in0=thz, scalar=1.0, in1=dgfA,
                                               op0=Alu.add, op1=Alu.mult)
                B1 = work.tile([H, Bc], f32, tag="B1")
                nc.gpsimd.tensor_scalar(out=B1, in0=thz, scalar1=-0.5, scalar2=0.5,
                                        op0=Alu.mult, op1=Alu.add)
            thn = work.tile([H, Bc], f32, tag="thn")
            nc.scalar.activation(thn, pren, Act.Tanh, bias=0.0, scale=1.0)
            new_h = work.tile([H, Bc], f32, tag="new_h")
            if dgfA is not None:
                tmp = work.tile([H, Bc], f32, tag="nhtmp")
                nc.vector.tensor_tensor(out=tmp, in0=B1, in1=thn, op=Alu.mult)
                nc.vector.tensor_tensor(out=new_h, in0=tmp, in1=A1, op=Alu.add)
            else:
                dgf = work.tile([H, Bc], f32, tag="dgf")
                nc.vector.scalar_tensor_tensor(out=dgf, in0=whp, scalar=0.25, in1=thn,
                                               op0=Alu.mult, op1=Alu.subtract)
                e = work.tile([H, Bc], f32, tag="e")
                nc.vector.scalar_tensor_tensor(out=e, in0=thz, scalar=1.0, in1=dgf,
                                               op0=Alu.add, op1=Alu.mult)
                nc.vector.scalar_tensor_tensor(out=new_h, in0=e, scalar=0.5, in1=thn,
                                               op0=Alu.mult, op1=Alu.add)
            return new_h

        def emit_fusion(t, new_h):
            pout = pout_sl
            nc.tensor.matmul(pout, sb["fuswhT"], new_h, start=True, stop=True)
            ob = t // OUT_BLK
            if ob not in out_tiles:
                out_tiles[ob] = out_sb_pool.tile([H, OUT_BLK, Bc], f32, tag="osb", name=f"osb_{ob}")
            nc.vector.tensor_tensor(out=out_tiles[ob][:, t % OUT_BLK, :], in0=pout,
                                     in1=fus_static, op=Alu.add)
            if t % OUT_BLK == OUT_BLK - 1:
                nc.sync.dma_start(out=out_d[:, t - OUT_BLK + 1:t + 1, :], in_=out_tiles[ob])
                del out_tiles[ob]

        # ---------- prologue ----------
        emit_phaseA(0)
        emit_phaseA(1)
        emit_phaseA(2)
        emit_gi_block(0)
        emit_gi_block(1)

        # fus_static = fuswsT@staticT + fusb
        pfs = pfs_sl
        nc.tensor.matmul(pfs, sb["fusb_row"], sb["ones_row"][:, 0:Bc], start=True, stop=False)
        nc.tensor.matmul(pfs, sb["fuswsT"], sb["staticT"], start=False, stop=True)
        nc.scalar.copy(fus_static, pfs)

        # t = 0: wh' = 4*cur0
        pinit = pinit_sl
        nc.tensor.matmul(pinit, sb["initb_row"], sb["ones_row"][:, 0:Bc], start=True, stop=False)
        nc.tensor.matmul(pinit, sb["initwT"], sb["staticT"], start=False, stop=True)
        def emit_maintenance(t):
            """After new_h_t (cur_h): pu1_pre(t+1), rsum update, ring_m write.
            pu1_pre mms carry no new_h dependency, so the close-mm of step
            t+1 is the only PE op on the recurrence cycle."""
            s2 = t % A
            if t + 1 < Tn:
                nc.tensor.matmul(pu1_sl, sb["w1sT"], sb["staticT"], start=True, stop=False)
                nc.tensor.matmul(pu1_sl, sb["w1h10T"], rsum, start=False, stop=False)
            nc.gpsimd.tensor_tensor(out=rsum, in0=rsum, in1=cur_h, op=Alu.add)
            s3 = (t + 1) % A
            if t + 1 < Tn:
                nc.gpsimd.tensor_tensor(out=rsum, in0=rsum, in1=ring_m[:, s3, :],
                                        op=Alu.subtract)
            nc.gpsimd.tensor_copy(ring_m[:, s2, :], cur_h)

        def emit_ring_transpose(t, h_t):
            nc.tensor.matmul(ptr_sl, h_t, sb["ident"],
                             is_transpose=True, start=True, stop=True)
            emit_fusion(t, h_t)

        def emit_ring_copy(t):
            s2 = t % A
            half2, sq2w = s2 % 2, s2 // 2
            if half2 == 0:
                nc.scalar.copy(ring2[0:Bc, sq2w, :], ptr_sl)
            else:
                hT = work.tile([Bc, H], f32, tag="hT")
                nc.scalar.copy(hT, ptr_sl)
                nc.sync.dma_start(out=ring2[Bc:2 * Bc, sq2w, :], in_=hT)

        wh0 = work.tile([H, Bc], f32, tag="wh0")
        nc.vector.tensor_scalar(out=wh0, in0=pinit, scalar1=4.0, scalar2=None, op0=Alu.mult)
        cur_h = emit_step_tail(0, wh0, wh0)
        emit_maintenance(0)

        # ---------- scan t = 1..Tn-1 ----------
        for t in range(1, Tn):
            blk, ti = t // 4, t % 4
            s = (t - 1) % A
            tt = t % 10

            # on-path: close the pu1 accumulation with the h_{t-1} term
            pu1 = pu1_sl
            nc.tensor.matmul(pu1, sb["w1h10T"], cur_h, start=False, stop=True)
            emit_ring_transpose(t - 1, cur_h)
            u1t = work.tile([U, Bc], f32, tag="u1t")
            nc.scalar.activation(u1t, pu1, Act.Tanh, bias=sb["a1b1"], scale=1.0)
            emit_ring_copy(t - 1)

            # logits1 in col-doubled layout [64, 20]; both 10-blocks hold all
            # 10 actions (permuted), so one row-max covers both.
            pl1 = pl1_sl
            for hf in range(2):
                nc.tensor.matmul(pl1[:, hf * A:(hf + 1) * A], sb["ones_row"][:, 0:Bc],
                                 sb["b2re1"][:, tt, hf, :], start=(hf == 0), stop=False)
                nc.tensor.matmul(pl1[:, hf * A:(hf + 1) * A], u1t,
                                 sb["w2re1"][:, tt, hf, :], start=False, stop=(hf == 1))
            rmax1 = work.tile([Bc, 1], f32, tag="rmax1")
            nc.vector.tensor_reduce(out=rmax1, in_=pl1, axis=Axis.X, op=Alu.max)
            cmask = work.tile([Bc, 2 * A], f32, tag="cmask")
            nc.vector.scalar_tensor_tensor(out=cmask, in0=pl1, scalar=rmax1[:, 0:1],
                                           in1=pre2_tiles[blk][:, ti, :],
                                           op0=Alu.is_ge, op1=Alu.add)
            # spread the per-half gather columns to partitions (half, b)
            cmd = cmd_sl
            nc.tensor.matmul(cmd, sb["SPL"], cmask[:, 0:5], start=True, stop=False)
            nc.tensor.matmul(cmd, sb["SPU"], cmask[:, A:A + 5], start=False, stop=True)
            # D[p, (s', b)] = cmd[p, s'] * eye2[p, b], then
            # wh'[h, b] = sum_s' ring2[:, :, s'].T @ D[:, s', :]
            Dm = work.tile([2 * Bc, 5, Bc], f32, tag="Dm")
            cm_b = bass.AP(tensor=cmd.tensor, offset=cmd.offset,
                           ap=[cmd.ap[0], [cmd.ap[1][0], 5], [0, Bc]])
            e2_b = bass.AP(tensor=sb["E2"].tensor, offset=sb["E2"].offset,
                           ap=[sb["E2"].ap[0], [0, 5], [sb["E2"].ap[1][0], Bc]])
            nc.vector.tensor_tensor(out=Dm, in0=cm_b, in1=e2_b, op=Alu.mult)
            pwh = pwh_sl
            for sq2 in range(5):
                nc.tensor.matmul(pwh, ring2[:, sq2, :], Dm[:, sq2, :],
                                 start=(sq2 == 0), stop=(sq2 == 4))
            whp_sb = work.tile([H, Bc], f32, tag="whp_sb")
            nc.vector.tensor_copy(whp_sb, pwh)
            dgfA = work.tile([H, Bc], f32, tag="dgfA")
            nc.gpsimd.tensor_scalar(out=dgfA, in0=whp_sb, scalar1=0.125, scalar2=None,
                                    op0=Alu.mult)

            cur_h = emit_step_tail(t, pwh, whp_sb, dgfA)
            emit_maintenance(t)
            if t == Tn - 1:
                emit_ring_transpose(t, cur_h)
                emit_ring_copy(t)
            # bulk blocks emitted after the chain so they fill PE gaps,
            # staggered across two steps to shorten each PE burst
            if ti == 1 and (blk + 1) * GI_BLK < Tn:
                emit_gi_block(blk + 1)
            if ti == 3:
                if (blk + 2) * PA_BLK < Tn:
                    emit_phaseA(blk + 2)
                if blk - 2 in pre2_tiles:
                    del pre2_tiles[blk - 2]

    nc.compile()
    _BUILD_CACHE[key] = (nc, "out")
    return _BUILD_CACHE[key]


def _prep_core_inputs(inputs, core, Tn=T, Bc=BC):
    f = np.float32
    b0 = core * Bc
    x = np.ascontiguousarray(inputs["x"][b0:b0 + Bc, :Tn, :]).astype(f)
    xT = np.ascontiguousarray(x.transpose(2, 1, 0).reshape(2, 128, Tn, Bc))
    staticT = np.ascontiguousarray(inputs["static"][b0:b0 + Bc].T).astype(f)
    wih = inputs["gru_wih"].astype(f); whh = inputs["gru_whh"].astype(f)
    a1w1 = inputs["a1_w1"].astype(f); a2w1 = inputs["a2_w1"].astype(f)
    bih = inputs["gru_bih"].astype(f); bhh = inputs["gru_bhh"].astype(f)
    w2_1 = inputs["a1_w2"].astype(f); w2_2 = inputs["a2_w2"].astype(f)
    b2_1 = inputs["a1_b2"].astype(f); b2_2 = inputs["a2_b2"].astype(f)

    # permuted second-layer weights: col j of variant (tt, half) = row a of w2
    # with a = (phys(j, half) - tt) mod 10
    w2re1 = np.zeros((10, 2, U, A), f); w2re2 = np.zeros((10, 2, U, A), f)
    b2re1 = np.zeros((10, 2, A), f); b2re2 = np.zeros((10, 2, A), f)
    for tt in range(10):
        for hf in range(2):
            for j in range(A):
                a = (_phys(j, hf) - tt) % 10
                w2re1[tt, hf, :, j] = w2_1[a, :]
                w2re2[tt, hf, :, j] = w2_2[a, :]
                b2re1[tt, hf, j] = b2_1[a]
                b2re2[tt, hf, j] = b2_2[a]

    # deltas: per phase-A block variant v (t0%10 = 2v cycle {0,8,6,4,2}),
    # 8 step-columns; each step t marks column j where phys(j, half) == (t-1)%10
    # with 2.0 (the 0.5*cur_h direct term, x4 scale)
    deltas = np.zeros((5, Bc, 4 * 2 * A), f)
    for blk_v in range(5):
        t0mod = (blk_v * 4) % 10
        for ti in range(4):
            tmod = (t0mod + ti) % 10
            s9 = (tmod - 1) % 10
            for hf in range(2):
                for j in range(A):
                    if _phys(j, hf) == s9:
                        deltas[blk_v, :, ti * 2 * A + hf * A + j] = 2.0

    m = {
        "xT": xT,
        "staticT": staticT,
        "static_rep": np.tile(staticT, (1, 8)),
        "wihT": np.ascontiguousarray(wih.T.reshape(2, 128, G3)),
        "whh4T": np.ascontiguousarray((whh / 4.0).T),
        "w1h10T": np.ascontiguousarray((a1w1[:, :H] / 10.0).T),
        "w1h10Tn": np.ascontiguousarray((-a1w1[:, :H] / 10.0).T),
        "w1sT": np.ascontiguousarray(a1w1[:, H:].T),
        "a2w1xT": np.ascontiguousarray(a2w1[:, :D].T.reshape(2, 128, U)),
        "a2w1sT": np.ascontiguousarray(a2w1[:, D:].T),
        "w2re1": w2re1, "w2re2": w2re2, "b2re1": b2re1, "b2re2": b2re2,
        "deltas": deltas,
        "initwT": np.ascontiguousarray(inputs["init_w"].astype(f).T),
        "fuswhT": np.ascontiguousarray(inputs["fus_w"].astype(f)[:, :H].T),
        "fuswsT": np.ascontiguousarray(inputs["fus_w"].astype(f)[:, H:].T),
        "a1b1": inputs["a1_b1"].astype(f).reshape(U, 1),
        "a2b1": inputs["a2_b1"].astype(f).reshape(U, 1),
        "brz_row": (bih[:2 * H] + bhh[:2 * H]).reshape(1, 2 * H).copy(),
        "bin_row": bih[2 * H:].reshape(1, H).copy(),
        "bhn_row": bhh[2 * H:].reshape(1, H).copy(),
        "ones_row": np.ones((1, 8 * Bc), f),
        "fusb_row": inputs["fus_b"].astype(f).reshape(1, H).copy(),
        "initb_row": inputs["init_b"].astype(f).reshape(1, H).copy(),
        "E2": np.tile(np.eye(Bc, dtype=f), (2, 1)),
        "SPL": np.concatenate([np.eye(Bc, dtype=f), np.zeros((Bc, Bc), f)], axis=1),
        "SPU": np.concatenate([np.zeros((Bc, Bc), f), np.eye(Bc, dtype=f)], axis=1),
        "ident": np.eye(128, dtype=f),
    }
    return {k: np.ascontiguousarray(v, dtype=f) for k, v in m.items()}


def kernel(**inputs):
    from concourse.bass_utils import run_bass_kernel_spmd
    nc, _ = _build(T, BC)
    in_maps = [_prep_core_inputs(inputs, c) for c in range(NCORES)]
    res = run_bass_kernel_spmd(nc, in_maps, core_ids=list(range(NCORES)))
    out = np.empty((B, T, H), np.float32)
    for c in range(NCORES):
        oc = res.results[c]["out"]
        out[c * BC:(c + 1) * BC] = oc.transpose(2, 1, 0)
    return out


# revision 7
# speedup vs baseline: 1.1999x; 1.0315x over previous
"""Trainium2 Bass kernel v2 for the AgentLayer GRU-with-action-memory model.

Data-parallel over batch: B=512 -> 8 cores x Bc=64. Per-core redesign vs v1:
  - gi (wih@x) computed in bulk 8-step blocks into PSUM banks; per-step gh
    matmuls accumulate into the same bank slices (skip_group_check).
  - argmax masks via is_ge against the row max (gaps verified > 2.7e-7 on
    this seed, so the max is unique); logits computed in a "doubled" layout
    [128 = 2 half-copies of b, 10 cols in per-half physical-slot order].
  - gather = one DVE/Pool masked-multiply over the ring
    [(slot%2)*64+b, h, slot//2] + 5 accumulating PE matmuls against a
    constant tiled-identity E2, producing weighted_h' = 4*weighted_h in
    [H, Bc] PSUM directly.  whh is pre-divided by 4 (exact) to compensate.
  - All matmuls strictly fp32 (fp32r/bf16 flip argmax decisions; verified).
"""

import numpy as np
from contextlib import ExitStack

B, T, D, H, S, A, U = 512, 256, 256, 128, 64, 10, 64
NCORES = 8
BC = B // NCORES  # 64
G3 = 3 * H        # 384

_BUILD_CACHE = {}


def _phys(j, upper):
    if j < 5:
        return 2 * j + (1 if upper else 0)
    return 2 * (j - 5) + (0 if upper else 1)


def _build(Tn, Bc):
    key = (Tn, Bc)
    if key in _BUILD_CACHE:
        return _BUILD_CACHE[key]

    import concourse.bass as bass
    import concourse.bacc as bacc
    import concourse.tile as tile
    from concourse import mybir

    f32 = mybir.dt.float32
    Alu = mybir.AluOpType
    Act = mybir.ActivationFunctionType
    Axis = mybir.AxisListType

    nc = bacc.Bacc("TRN2", target_bir_lowering=False, debug=False)

    d_in = {}

    def din(name, shape):
        d_in[name] = nc.dram_tensor(name, list(shape), f32, kind="ExternalInput").ap()
        return d_in[name]

    xT = din("xT", (2, 128, Tn, Bc))          # x as [chunk, d, t, b]
    staticT = din("staticT", (S, Bc))
    static_rep = din("static_rep", (S, 8 * Bc))
    wihT = din("wihT", (2, 128, G3))
    whh4T = din("whh4T", (H, G3))             # (whh/4).T
    w1h10T = din("w1h10T", (H, U))            # (a1_w1[:, :H]/10).T
    w1h10Tn = din("w1h10Tn", (H, U))          # negated
    w1sT = din("w1sT", (S, U))
    a2w1xT = din("a2w1xT", (2, 128, U))
    a2w1sT = din("a2w1sT", (S, U))
    w2re1 = din("w2re1", (10, 2, U, A))       # a1_w2.T column-permuted per t%10, half
    w2re2 = din("w2re2", (10, 2, U, A))       # a2_w2.T likewise
    b2re1 = din("b2re1", (10, 2, A))          # a1_b2 permuted rows
    b2re2 = din("b2re2", (10, 2, A))
    deltas = din("deltas", (5, 2 * Bc, 4 * A))  # phase-A marks, doubled partitions
    initwT = din("initwT", (S, H))
    fuswhT = din("fuswhT", (H, H))
    fuswsT = din("fuswsT", (S, H))
    a1b1 = din("a1b1", (U, 1))
    a2b1 = din("a2b1", (U, 1))
    brz_row = din("brz_row", (1, 2 * H))      # bih+bhh for r,z gates
    bin_row = din("bin_row", (1, H))          # bih n-gate
    bhn_row = din("bhn_row", (1, H))          # bhh n-gate
    ones_row = din("ones_row", (1, 8 * Bc))
    fusb_row = din("fusb_row", (1, H))
    initb_row = din("initb_row", (1, H))
    E2 = din("E2", (2 * Bc, Bc))              # tiled identity
    SPL = din("SPL", (Bc, 2 * Bc))            # [eye | 0] spread-to-lower
    SPU = din("SPU", (Bc, 2 * Bc))            # [0 | eye] spread-to-upper
    onesL = din("onesL", (1, 2 * Bc))         # [1...1 | 0...0]
    onesU = din("onesU", (1, 2 * Bc))         # [0...0 | 1...1]
    ident = din("ident", (128, 128))

    out_d = nc.dram_tensor("out", [H, Tn, Bc], f32, kind="ExternalOutput").ap()

    NBLK = Tn // 8
    assert Tn % 8 == 0
    OUT_BLK = 16

    with ExitStack() as ctx:
        tc = ctx.enter_context(tile.TileContext(nc))
        singles = ctx.enter_context(tc.tile_pool(name="singles", bufs=1))
        work = ctx.enter_context(tc.tile_pool(name="work", bufs=3))
        psg = ctx.enter_context(tc.tile_pool(name="psg", bufs=2, space="PSUM"))     # gi banks (3/blk)
        psfix = ctx.enter_context(tc.tile_pool(name="psfix", bufs=1, space="PSUM"))  # 2 packed banks
        u2t_pool = ctx.enter_context(tc.tile_pool(name="u2t", bufs=2))
        pre2_pool = ctx.enter_context(tc.tile_pool(name="pre2", bufs=4))
        out_sb_pool = ctx.enter_context(tc.tile_pool(name="outsb", bufs=2))

        # ---- load constants / weights into SBUF ----
        sb = {}
        for name, ap in d_in.items():
            if name == "xT":
                continue
            if name in ("wihT", "a2w1xT"):
                t = singles.tile([128, 2, ap.shape[2]], f32, tag=f"w_{name}", name=f"w_{name}")
                for c in range(2):
                    nc.sync.dma_start(out=t[:, c, :], in_=ap[c])
            elif name in ("w2re1", "w2re2"):
                t = singles.tile([U, 10, 2, A], f32, tag=f"w_{name}", name=f"w_{name}")
                nc.sync.dma_start(out=t, in_=bass.AP(
                    tensor=ap.tensor, offset=ap.offset,
                    ap=[[ap.ap[2][0], U], [ap.ap[0][0], 10], [ap.ap[1][0], 2], [ap.ap[3][0], A]]))
            elif name in ("b2re1", "b2re2"):
                t = singles.tile([1, 10, 2, A], f32, tag=f"w_{name}", name=f"w_{name}")
                nc.sync.dma_start(out=t, in_=bass.AP(
                    tensor=ap.tensor, offset=ap.offset,
                    ap=[[0, 1], [ap.ap[0][0], 10], [ap.ap[1][0], 2], [ap.ap[2][0], A]]))
            elif name == "deltas":
                t = singles.tile([2 * Bc, 5, 4 * A], f32, tag="w_deltas", name="w_deltas")
                nc.sync.dma_start(out=t, in_=bass.AP(
                    tensor=ap.tensor, offset=ap.offset,
                    ap=[[ap.ap[1][0], 2 * Bc], [ap.ap[0][0], 5], [ap.ap[2][0], 4 * A]]))
            else:
                t = singles.tile(list(ap.shape), f32, tag=f"w_{name}", name=f"w_{name}")
                nc.sync.dma_start(out=t, in_=ap)
            sb[name] = t

        xsb = singles.tile([128, 2, Tn, Bc], f32, tag="xsb")
        for c in range(2):
            nc.sync.dma_start(out=xsb[:, c, :, :], in_=xT[c])

        # ---- persistent state ----
        ring2 = singles.tile([2 * Bc, 5, H], f32, tag="ring2")   # [(s%2)*64+b, s//2, h]
        ring_m = singles.tile([H, A, Bc], f32, tag="ring_m")     # [h, slot, b]
        rsum = singles.tile([H, Bc], f32, tag="rsum")
        u1tw = singles.tile([U, 3, Bc], f32, tag="u1tw")   # [0|u1t|0]
        nc.vector.memset(u1tw, 0.0)
        nc.vector.memset(ring2, 0.0)
        nc.vector.memset(ring_m, 0.0)
        nc.vector.memset(rsum, 0.0)

        pre2_tiles = {}

        # two packed psum banks, manually partitioned by column range
        psA_t = psfix.tile([128, 512], f32, tag="psA")
        psB_t = psfix.tile([128, 512], f32, tag="psB")
        pu2_sl = psA_t[0:U, 0:256]
        pl2_sl = psA_t[0:Bc, 256:336].rearrange("p (t a) -> p t a", a=2 * A)
        ptr_sl = psA_t[0:Bc, 336:464]
        pu1_sl = psB_t[0:U, 0:Bc]
        pl1_sl = psB_t[:, 64:64 + A]
        pre2d_sl = psA_t[:, 464:504].rearrange("p (t a) -> p t a", a=A)
        pwh_sl = psB_t[:, 128:192]
        phn_sl = psB_t[:, 192:256]
        pout_sl = psB_t[:, 256:320]
        pfs_sl = psB_t[:, 320:384]
        pinit_sl = psB_t[:, 384:448]

        # ---------- phase A: pre2_d for a 4-step block ----------
        PA_BLK = 4
        def emit_phaseA(blk):
            t0 = blk * PA_BLK
            NF = PA_BLK * Bc
            pu2 = pu2_sl
            for c in range(2):
                nc.tensor.matmul(pu2, sb["a2w1xT"][:, c, :],
                                 xsb[:, c, t0:t0 + PA_BLK, :].rearrange("p t b -> p (t b)"),
                                 start=(c == 0), stop=False)
            nc.tensor.matmul(pu2, sb["a2w1sT"], sb["static_rep"][:, 0:NF],
                             start=False, stop=True)
            u2t = u2t_pool.tile([U, NF], f32, tag="u2t")
            nc.scalar.activation(u2t, pu2, Act.Tanh, bias=sb["a2b1"], scale=1.0)
            pl2 = pl2_sl
            for ti in range(PA_BLK):
                tt = (t0 + ti) % 10
                for half in range(2):
                    nc.tensor.matmul(pl2[:, ti, half * A:(half + 1) * A],
                                     sb["ones_row"][:, 0:Bc], sb["b2re2"][:, tt, half, :],
                                     start=(half == 0), stop=False)
                    nc.tensor.matmul(pl2[:, ti, half * A:(half + 1) * A],
                                     u2t[:, ti * Bc:(ti + 1) * Bc],
                                     sb["w2re2"][:, tt, half, :],
                                     start=False, stop=(half == 1))
            rmax2 = work.tile([Bc, PA_BLK], f32, tag="rmax2")
            nc.vector.tensor_reduce(out=rmax2, in_=pl2, axis=Axis.X, op=Alu.max)
            rmax2_b = bass.AP(tensor=rmax2.tensor, offset=rmax2.offset,
                              ap=[rmax2.ap[0], rmax2.ap[1], [0, 2 * A]])
            ge2 = work.tile([Bc, PA_BLK, 2 * A], f32, tag="ge2")
            nc.vector.tensor_tensor(out=ge2, in0=pl2, in1=rmax2_b, op=Alu.is_ge)
            # spread to doubled partitions [(hf,b), ti, 10] + fold the delta
            for ti in range(PA_BLK):
                nc.tensor.matmul(pre2d_sl[:, ti, :], sb["SPL"], ge2[:, ti, 0:A],
                                 start=True, stop=False)
                nc.tensor.matmul(pre2d_sl[:, ti, :], sb["SPU"], ge2[:, ti, A:2 * A],
                                 start=False, stop=True)
            pre2 = pre2_pool.tile([2 * Bc, PA_BLK, A], f32, tag="pre2", name=f"pre2_{blk}")
            nc.vector.tensor_tensor(
                out=pre2, in0=pre2d_sl,
                in1=sb["deltas"][:, blk % 5, :].rearrange("p (t a) -> p t a", a=A),
                op=Alu.add)
            pre2_tiles[blk] = pre2

        # ---------- bulk gi for a 4-step block (GI_BLK=4 keeps each psum
        # tile within one 2KB bank) ----------
        GI_BLK = 4
        gi_banks = {}

        def emit_gi_block(gblk):
            t0 = gblk * GI_BLK
            NF = GI_BLK * Bc
            gr = psg.tile([128, GI_BLK, Bc], f32, tag="gir", name=f"gir_{gblk}")
            gz = psg.tile([128, GI_BLK, Bc], f32, tag="giz", name=f"giz_{gblk}")
            gn = psg.tile([128, GI_BLK, Bc], f32, tag="gin", name=f"gin_{gblk}")
            for gate, tile_ in ((0, gr), (1, gz), (2, gn)):
                flat = tile_.rearrange("p t b -> p (t b)")
                brow = (sb["brz_row"][:, gate * H:(gate + 1) * H] if gate < 2
                        else sb["bin_row"])
                nc.tensor.matmul(flat, brow, sb["ones_row"][:, 0:NF],
                                 start=True, stop=False, skip_group_check=True)
                for c in range(2):
                    nc.tensor.matmul(flat, sb["wihT"][:, c, gate * H:(gate + 1) * H],
                                     xsb[:, c, t0:t0 + GI_BLK, :].rearrange("p t b -> p (t b)"),
                                     start=False, stop=False, skip_group_check=True)
            gi_banks[gblk] = (gr, gz, gn)

        # ---------- per-step tail: gh, gates, fusion ----------
        out_tiles = {}
        fus_static = singles.tile([H, Bc], f32, tag="fus_static")

        def emit_step_tail(t, whp, whp_sb, dgfA=None):
            """whp: psum (or sbuf) [H, Bc] holding 4*weighted_h; whp_sb: sbuf copy."""
            gblk, ti = t // GI_BLK, t % GI_BLK
            gr, gz, gn = gi_banks[gblk]
            # gh accumulation into gi slices
            nc.tensor.matmul(gr[:, ti, :], sb["whh4T"][:, 0:H], whp_sb,
                             start=False, stop=True, skip_group_check=True)
            phn = phn_sl
            nc.tensor.matmul(phn, sb["bhn_row"], sb["ones_row"][:, 0:Bc], start=True, stop=False)
            nc.tensor.matmul(phn, sb["whh4T"][:, 2 * H:3 * H], whp_sb, start=False, stop=True)
            nc.tensor.matmul(gz[:, ti, :], sb["whh4T"][:, H:2 * H], whp_sb,
                             start=False, stop=True, skip_group_check=True)
            thr = work.tile([H, Bc], f32, tag="thr")
            nc.scalar.activation(thr, gr[:, ti, :], Act.Tanh, bias=0.0, scale=0.5)
            thz = work.tile([H, Bc], f32, tag="thz")
            nc.scalar.activation(thz, gz[:, ti, :], Act.Tanh, bias=0.0, scale=0.5)
            q = work.tile([H, Bc], f32, tag="q")
            nc.vector.scalar_tensor_tensor(out=q, in0=thr, scalar=1.0, in1=phn,
                                           op0=Alu.add, op1=Alu.mult)
            pren = work.tile([H, Bc], f32, tag="pren")
            nc.vector.scalar_tensor_tensor(out=pren, in0=q, scalar=0.5, in1=gn[:, ti, :],
                                           op0=Alu.mult, op1=Alu.add)
            # new_h = 0.5(1+thz)*dgfA' + 0.5(1-thz)*thn  with dgfA' = 0.25wh'
            # (dgfA here = 0.125*wh' so A1 = 0.5(1+thz)*0.25wh').  A1/B1 need
            # only thz, so they run while thn's tanh is still in flight.
            if dgfA is not None:
                A1 = work.tile([H, Bc], f32, tag="A1")
                nc.vector.scalar_tensor_tensor(out=A1, in0=thz, scalar=1.0, in1=dgfA,
                                               op0=Alu.add, op1=Alu.mult)
                B1 = work.tile([H, Bc], f32, tag="B1")
                nc.gpsimd.tensor_scalar(out=B1, in0=thz, scalar1=-0.5, scalar2=0.5,
                                        op0=Alu.mult, op1=Alu.add)
            thn = work.tile([H, Bc], f32, tag="thn")
            nc.scalar.activation(thn, pren, Act.Tanh, bias=0.0, scale=1.0)
            new_h = work.tile([H, Bc], f32, tag="new_h")
            if dgfA is not None:
                tmp = work.tile([H, Bc], f32, tag="nhtmp")
                nc.vector.tensor_tensor(out=tmp, in0=B1, in1=thn, op=Alu.mult)
                nc.vector.tensor_tensor(out=new_h, in0=tmp, in1=A1, op=Alu.add)
            else:
                dgf = work.tile([H, Bc], f32, tag="dgf")
                nc.vector.scalar_tensor_tensor(out=dgf, in0=whp, scalar=0.25, in1=thn,
                                               op0=Alu.mult, op1=Alu.subtract)
                e = work.tile([H, Bc], f32, tag="e")
                nc.vector.scalar_tensor_tensor(out=e, in0=thz, scalar=1.0, in1=dgf,
                                               op0=Alu.add, op1=Alu.mult)
                nc.vector.scalar_tensor_tensor(out=new_h, in0=e, scalar=0.5, in1=thn,
                                               op0=Alu.mult, op1=Alu.add)
            return new_h

        def emit_fusion(t, new_h):
            pout = pout_sl
            nc.tensor.matmul(pout, sb["fuswhT"], new_h, start=True, stop=True)
            ob = t // OUT_BLK
            if ob not in out_tiles:
                out_tiles[ob] = out_sb_pool.tile([H, OUT_BLK, Bc], f32, tag="osb", name=f"osb_{ob}")
            nc.vector.tensor_tensor(out=out_tiles[ob][:, t % OUT_BLK, :], in0=pout,
                                     in1=fus_static, op=Alu.add)
            if t % OUT_BLK == OUT_BLK - 1:
                nc.sync.dma_start(out=out_d[:, t - OUT_BLK + 1:t + 1, :], in_=out_tiles[ob])
                del out_tiles[ob]

        # ---------- prologue ----------
        emit_phaseA(0)
        emit_phaseA(1)
        emit_phaseA(2)
        emit_gi_block(0)
        emit_gi_block(1)

        # fus_static = fuswsT@staticT + fusb
        pfs = pfs_sl
        nc.tensor.matmul(pfs, sb["fusb_row"], sb["ones_row"][:, 0:Bc], start=True, stop=False)
        nc.tensor.matmul(pfs, sb["fuswsT"], sb["staticT"], start=False, stop=True)
        nc.scalar.copy(fus_static, pfs)

        # t = 0: wh' = 4*cur0
        pinit = pinit_sl
        nc.tensor.matmul(pinit, sb["initb_row"], sb["ones_row"][:, 0:Bc], start=True, stop=False)
        nc.tensor.matmul(pinit, sb["initwT"], sb["staticT"], start=False, stop=True)
        def emit_maintenance(t):
            """After new_h_t (cur_h): pu1_pre(t+1), rsum update, ring_m write.
            pu1_pre mms carry no new_h dependency, so the close-mm of step
            t+1 is the only PE op on the recurrence cycle."""
            s2 = t % A
            if t + 1 < Tn:
                nc.tensor.matmul(pu1_sl, sb["w1sT"], sb["staticT"], start=True, stop=False)
                nc.tensor.matmul(pu1_sl, sb["w1h10T"], rsum, start=False, stop=False)
            nc.gpsimd.tensor_tensor(out=rsum, in0=rsum, in1=cur_h, op=Alu.add)
            s3 = (t + 1) % A
            if t + 1 < Tn:
                nc.gpsimd.tensor_tensor(out=rsum, in0=rsum, in1=ring_m[:, s3, :],
                                        op=Alu.subtract)
            nc.gpsimd.tensor_copy(ring_m[:, s2, :], cur_h)

        def emit_ring_transpose(t, h_t):
            nc.tensor.matmul(ptr_sl, h_t, sb["ident"],
                             is_transpose=True, start=True, stop=True)
            emit_fusion(t, h_t)

        def emit_ring_copy(t):
            s2 = t % A
            half2, sq2w = s2 % 2, s2 // 2
            if half2 == 0:
                nc.scalar.copy(ring2[0:Bc, sq2w, :], ptr_sl)
            else:
                hT = work.tile([Bc, H], f32, tag="hT")
                nc.scalar.copy(hT, ptr_sl)
                nc.sync.dma_start(out=ring2[Bc:2 * Bc, sq2w, :], in_=hT)

        wh0 = work.tile([H, Bc], f32, tag="wh0")
        nc.vector.tensor_scalar(out=wh0, in0=pinit, scalar1=4.0, scalar2=None, op0=Alu.mult)
        cur_h = emit_step_tail(0, wh0, wh0)
        emit_maintenance(0)

        # ---------- scan t = 1..Tn-1 ----------
        for t in range(1, Tn):
            blk, ti = t // 4, t % 4
            s = (t - 1) % A
            tt = t % 10

            # on-path: close the pu1 accumulation with the h_{t-1} term
            pu1 = pu1_sl
            nc.tensor.matmul(pu1, sb["w1h10T"], cur_h, start=False, stop=True)
            emit_ring_transpose(t - 1, cur_h)
            # u1t written into the middle of [0|u1t|0]; the two matmul lhsT
            # views [64:192] = [u1t|0] and [0:128] = [0|u1t] share it
            nc.scalar.activation(u1tw[:, 1, :], pu1, Act.Tanh, bias=sb["a1b1"], scale=1.0)
            emit_ring_copy(t - 1)

            # doubled logits1 [128, 10] via zero-padded lhsT matmuls (both
            # write the full base-0 tile; zeros land on the other half)
            pl1 = pl1_sl
            nc.tensor.matmul(pl1, sb["onesL"], sb["b2re1"][:, tt, 0, :],
                             start=True, stop=False)
            nc.tensor.matmul(pl1, sb["onesU"], sb["b2re1"][:, tt, 1, :],
                             start=False, stop=False)
            nc.tensor.matmul(pl1, u1tw.rearrange("p c b -> p (c b)")[:, Bc:3 * Bc],
                             sb["w2re1"][:, tt, 0, :], start=False, stop=False)
            nc.tensor.matmul(pl1, u1tw.rearrange("p c b -> p (c b)")[:, 0:2 * Bc],
                             sb["w2re1"][:, tt, 1, :], start=False, stop=True)
            rmax1 = work.tile([2 * Bc, 1], f32, tag="rmax1")
            nc.vector.tensor_reduce(out=rmax1, in_=pl1, axis=Axis.X, op=Alu.max)
            cmask = work.tile([2 * Bc, A], f32, tag="cmask")
            nc.vector.scalar_tensor_tensor(out=cmask, in0=pl1, scalar=rmax1[:, 0:1],
                                           in1=pre2_tiles[blk][:, ti, :],
                                           op0=Alu.is_ge, op1=Alu.add)
            # D[p, (s', b)] = cmask[p, s'] * eye2[p, b]  (gather cols = 0:5)
            Dm = work.tile([2 * Bc, 5, Bc], f32, tag="Dm")
            cm_b = bass.AP(tensor=cmask.tensor, offset=cmask.offset,
                           ap=[cmask.ap[0], [cmask.ap[1][0], 5], [0, Bc]])
            e2_b = bass.AP(tensor=sb["E2"].tensor, offset=sb["E2"].offset,
                           ap=[sb["E2"].ap[0], [0, 5], [sb["E2"].ap[1][0], Bc]])
            nc.vector.tensor_tensor(out=Dm, in0=cm_b, in1=e2_b, op=Alu.mult)
            pwh = pwh_sl
            for sq2 in range(5):
                nc.tensor.matmul(pwh, ring2[:, sq2, :], Dm[:, sq2, :],
                                 start=(sq2 == 0), stop=(sq2 == 4))
            whp_sb = work.tile([H, Bc], f32, tag="whp_sb")
            nc.vector.tensor_copy(whp_sb, pwh)
            dgfA = work.tile([H, Bc], f32, tag="dgfA")
            nc.gpsimd.tensor_scalar(out=dgfA, in0=whp_sb, scalar1=0.125, scalar2=None,
                                    op0=Alu.mult)

            cur_h = emit_step_tail(t, pwh, whp_sb, dgfA)
            emit_maintenance(t)
            if t == Tn - 1:
                emit_ring_transpose(t, cur_h)
                emit_ring_copy(t)
            # bulk blocks emitted after the chain so they fill PE gaps,
            # staggered across two steps to shorten each PE burst
            if ti == 1 and (blk + 1) * GI_BLK < Tn:
                emit_gi_block(blk + 1)
            if ti == 3:
                if (blk + 2) * PA_BLK < Tn:
                    emit_phaseA(blk + 2)
                if blk - 2 in pre2_tiles:
                    del pre2_tiles[blk - 2]

    nc.compile()
    _BUILD_CACHE[key] = (nc, "out")
    return _BUILD_CACHE[key]


def _prep_core_inputs(inputs, core, Tn=T, Bc=BC):
    f = np.float32
    b0 = core * Bc
    x = np.ascontiguousarray(inputs["x"][b0:b0 + Bc, :Tn, :]).astype(f)
    xT = np.ascontiguousarray(x.transpose(2, 1, 0).reshape(2, 128, Tn, Bc))
    staticT = np.ascontiguousarray(inputs["static"][b0:b0 + Bc].T).astype(f)
    wih = inputs["gru_wih"].astype(f); whh = inputs["gru_whh"].astype(f)
    a1w1 = inputs["a1_w1"].astype(f); a2w1 = inputs["a2_w1"].astype(f)
    bih = inputs["gru_bih"].astype(f); bhh = inputs["gru_bhh"].astype(f)
    w2_1 = inputs["a1_w2"].astype(f); w2_2 = inputs["a2_w2"].astype(f)
    b2_1 = inputs["a1_b2"].astype(f); b2_2 = inputs["a2_b2"].astype(f)

    # permuted second-layer weights: col j of variant (tt, half) = row a of w2
    # with a = (phys(j, half) - tt) mod 10
    w2re1 = np.zeros((10, 2, U, A), f); w2re2 = np.zeros((10, 2, U, A), f)
    b2re1 = np.zeros((10, 2, A), f); b2re2 = np.zeros((10, 2, A), f)
    for tt in range(10):
        for hf in range(2):
            for j in range(A):
                a = (_phys(j, hf) - tt) % 10
                w2re1[tt, hf, :, j] = w2_1[a, :]
                w2re2[tt, hf, :, j] = w2_2[a, :]
                b2re1[tt, hf, j] = b2_1[a]
                b2re2[tt, hf, j] = b2_2[a]

    # deltas: per phase-A block variant v (t0%10 = 2v cycle {0,8,6,4,2}),
    # 8 step-columns; each step t marks column j where phys(j, half) == (t-1)%10
    # with 2.0 (the 0.5*cur_h direct term, x4 scale)
    deltas = np.zeros((5, 2 * Bc, 4 * A), f)
    for blk_v in range(5):
        t0mod = (blk_v * 4) % 10
        for ti in range(4):
            tmod = (t0mod + ti) % 10
            s9 = (tmod - 1) % 10
            for hf in range(2):
                for j in range(A):
                    if _phys(j, hf) == s9:
                        deltas[blk_v, hf * Bc:(hf + 1) * Bc, ti * A + j] = 2.0

    m = {
        "xT": xT,
        "staticT": staticT,
        "static_rep": np.tile(staticT, (1, 8)),
        "wihT": np.ascontiguousarray(wih.T.reshape(2, 128, G3)),
        "whh4T": np.ascontiguousarray((whh / 4.0).T),
        "w1h10T": np.ascontiguousarray((a1w1[:, :H] / 10.0).T),
        "w1h10Tn": np.ascontiguousarray((-a1w1[:, :H] / 10.0).T),
        "w1sT": np.ascontiguousarray(a1w1[:, H:].T),
        "a2w1xT": np.ascontiguousarray(a2w1[:, :D].T.reshape(2, 128, U)),
        "a2w1sT": np.ascontiguousarray(a2w1[:, D:].T),
        "w2re1": w2re1, "w2re2": w2re2, "b2re1": b2re1, "b2re2": b2re2,
        "deltas": deltas,
        "initwT": np.ascontiguousarray(inputs["init_w"].astype(f).T),
        "fuswhT": np.ascontiguousarray(inputs["fus_w"].astype(f)[:, :H].T),
        "fuswsT": np.ascontiguousarray(inputs["fus_w"].astype(f)[:, H:].T),
        "a1b1": inputs["a1_b1"].astype(f).reshape(U, 1),
        "a2b1": inputs["a2_b1"].astype(f).reshape(U, 1),
        "brz_row": (bih[:2 * H] + bhh[:2 * H]).reshape(1, 2 * H).copy(),
        "bin_row": bih[2 * H:].reshape(1, H).copy(),
        "bhn_row": bhh[2 * H:].reshape(1, H).copy(),
        "ones_row": np.ones((1, 8 * Bc), f),
        "fusb_row": inputs["fus_b"].astype(f).reshape(1, H).copy(),
        "initb_row": inputs["init_b"].astype(f).reshape(1, H).copy(),
        "E2": np.tile(np.eye(Bc, dtype=f), (2, 1)),
        "SPL": np.concatenate([np.eye(Bc, dtype=f), np.zeros((Bc, Bc), f)], axis=1),
        "SPU": np.concatenate([np.zeros((Bc, Bc), f), np.eye(Bc, dtype=f)], axis=1),
        "onesL": np.concatenate([np.ones((1, Bc), f), np.zeros((1, Bc), f)], axis=1),
        "onesU": np.concatenate([np.zeros((1, Bc), f), np.ones((1, Bc), f)], axis=1),
        "ident": np.eye(128, dtype=f),
    }
    return {k: np.ascontiguousarray(v, dtype=f) for k, v in m.items()}


def kernel(**inputs):
    from concourse.bass_utils import run_bass_kernel_spmd
    nc, _ = _build(T, BC)
    in_maps = [_prep_core_inputs(inputs, c) for c in range(NCORES)]
    res = run_bass_kernel_spmd(nc, in_maps, core_ids=list(range(NCORES)))
    out = np.empty((B, T, H), np.float32)
    for c in range(NCORES):
        oc = res.results[c]["out"]
        out[c * BC:(c + 1) * BC] = oc.transpose(2, 1, 0)
    return out
